# revision 7
# baseline (speedup 1.0000x reference)
"""OFA attention (dense_transformer) on 8 Trainium2 NeuronCores.

Sharding: heads split over cores (core c owns heads {2c, 2c+1}, both batches).
Per-core Bass/Tile program (build_attention_nc):
  phase 1 : QT/KT/VT = W_c @ hs.T (transposed projections; SCALING folded into Wq,
            c_attn folded into Wv on host; bias-add fused into PSUM drain on ScalarE)
  phase 1b: V natural = PE-transpose(VT), packed [V_A | 1 | V_B | 1] bf16
  phase 2 : per (batch, 512-token t-block), streaming 128-row s-tiles:
              ST(s,t) = K Q^T            (row-tiled K=64 matmuls)
              ST += bias.T               (bias pre-transposed to [s,t] bf16 on HOST;
                                          adds split: some s-tiles accumulate via a
                                          resident-identity matmul on PE, the rest on
                                          DVE tensor_add into an f32 staging tile)
              E = exp(ST)                (ScalarE only; exp without max-subtraction:
                                          scores+bias stay in [-8, 8])
              [O.T ; sums] += [V|1].T@E  (PV matmul also produces softmax denominators)
            per head: r = 1/sums (DVE), partition_broadcast (GpSimd), O.T drain fused
            with the r scaling (DVE tensor_mul) -> out-projection runs at full
            contract-128 (both heads in one matmul), spread across the next group's
            sp iterations; drains split DVE/GpSimd.
Host: partial outputs summed over cores + bo (the "all-reduce" of the out-projection).
"""
import sys

for _p in ("/opt/trn_rl_repo",):
    if _p not in sys.path:
        sys.path.append(_p)

import numpy as np

import concourse.bass as bass
import concourse.tile as tile
from concourse import mybir
from concourse.masks import make_identity
from concourse.bass_utils import run_bass_kernel_spmd

F32 = mybir.dt.float32
BF16 = mybir.dt.bfloat16

B, T, E, NH, D = 2, 2048, 1024, 16, 64
N_CORES = 8
HPC = NH // N_CORES
DH = HPC * D
SCALING = float(D * 2.0) ** -0.5

# Bias-add engine per unit u = sp*2 + a: 'p' = PE identity-matmul, 'v' = DVE.
# (GpSimd cannot read PSUM, so it only does broadcasts + bias DMA triggers.)
ASSIGN = ['p', 'v', 'v', 'p', 'v', 'v', 'p', 'v',
          'v', 'p', 'v', 'v', 'p', 'v', 'v', 'p']


def _waitfix(nc, limit=1):
    """This walrus build accepts at most ONE sync-wait per instruction.
    Hoist excess sem-waits onto inserted single-wait NoOps."""
    n_fixed = 0
    for bb in nc.m.functions[0].blocks:
        i = 0
        insts = bb.instructions
        while i < len(insts):
            inst = insts[i]
            si = inst.sync_info
            if si and si.on_wait and len(si.on_wait) > limit:
                extra = si.on_wait[limit:]
                si.on_wait = si.on_wait[:limit]
                for k, w in enumerate(extra):
                    nop = mybir.InstNoOp(
                        name=f"{inst.name}-waitfix{k}",
                        engine=inst.engine,
                        sync_info=mybir.SyncInfo(on_wait=[w], on_update=[]),
                        bass_nofuse=True,
                    )
                    nc.register_instruction(nop, overwrite=True)
                    insts.insert(i, nop)
                    i += 1
                n_fixed += 1
            i += 1
    return n_fixed


def build_attention_nc(B=2, T=2048, E=1024, HPC=2, D=64,
                       T_BLOCK=512, PROJ_BLOCK=512):
    """Build the per-core Bass program. Returns nc."""
    S = T
    PROJ_BLOCK = min(PROJ_BLOCK, T)
    TOK = B * T
    DH = HPC * D                      # 128
    assert DH == 128 and D == 64 and HPC == 2
    NE = E // 128                     # e-tiles
    NST = S // 128                    # s-tiles per batch
    NTB = T // T_BLOCK                # t-blocks per batch
    NJ = T_BLOCK // 128               # t-subtiles per block
    NPB = TOK // PROJ_BLOCK           # proj token blocks

    nc = bass.Bass()

    hsT = nc.declare_dram_parameter("hsT", [E, TOK], BF16, isOutput=False)
    wqT = nc.declare_dram_parameter("wqT", [E, DH], BF16, isOutput=False)
    wkT = nc.declare_dram_parameter("wkT", [E, DH], BF16, isOutput=False)
    wvT = nc.declare_dram_parameter("wvT", [E, DH], BF16, isOutput=False)
    bq = nc.declare_dram_parameter("bq", [DH, 1], F32, isOutput=False)
    bk = nc.declare_dram_parameter("bk", [DH, 1], F32, isOutput=False)
    bv = nc.declare_dram_parameter("bv", [DH, 1], F32, isOutput=False)
    woT = nc.declare_dram_parameter("woT", [DH, E], BF16, isOutput=False)
    biasT_in = nc.declare_dram_parameter("biasT", [B, HPC, S, T], BF16,
                                         isOutput=False)
    out_partial = nc.declare_dram_parameter("out", [TOK, E], BF16, isOutput=True)
    rc_dram = nc.dram_tensor("rc_scratch", [B * NTB * HPC, T_BLOCK], F32)

    with tile.TileContext(nc) as tc:
        from contextlib import ExitStack
        with ExitStack() as ctx:
            consts = ctx.enter_context(tc.tile_pool(name="consts", bufs=1))
            persist = ctx.enter_context(tc.tile_pool(name="persist", bufs=1))
            bias_pool = ctx.enter_context(tc.tile_pool(name="bias_sb", bufs=2))

            i_bf = consts.tile([128, 128], BF16, tag="i_bf")
            make_identity(nc, i_bf[:])

            # weights: (E, DH) -> (128, NE, DH), bf16
            w_sb = {}
            for name, src in (("wq", wqT), ("wk", wkT), ("wv", wvT)):
                t = consts.tile([128, NE, DH], BF16, tag=name)
                nc.sync.dma_start(out=t[:], in_=src.rearrange("(n p) d -> p n d", p=128))
                w_sb[name] = t
            wo_sb = consts.tile([128, E], BF16, tag="wo")
            nc.sync.dma_start(out=wo_sb[:], in_=woT[:, :])
            b_sb = {}
            for name, src in (("bq", bq), ("bk", bk), ("bv", bv)):
                t = consts.tile([128, 1], F32, tag=name)
                nc.sync.dma_start(out=t[:], in_=src[:, :])
                b_sb[name] = t

            # persistent activations
            QTb = [persist.tile([128, T], BF16, tag=f"QT{bb}", name=f"QT{bb}")
                   for bb in range(B)]
            KTb = [persist.tile([128, T], BF16, tag=f"KT{bb}", name=f"KT{bb}")
                   for bb in range(B)]
            VTb = [persist.tile([128, T], BF16, tag=f"VT{bb}", name=f"VT{bb}")
                   for bb in range(B)]
            V_sbb = []
            for bb in range(B):
                V_sb = persist.tile([128, T // 128, 256], BF16, tag=f"V_sb{bb}",
                                    name=f"V_sb{bb}")
                nc.vector.memset(V_sb[:, :, :], 0.0)
                nc.vector.memset(V_sb[:, :, D:D + 1], 1.0)
                nc.vector.memset(V_sb[:, :, 128 + D:128 + D + 1], 1.0)
                V_sbb.append(V_sb)

            # group list + bias prefetch plumbing (1-group emission lookahead;
            # DMAs ride the otherwise-idle GpSimd queue, transfers overlap the
            # previous group / phase 1)
            groups = [(b, tb) for b in range(B) for tb in range(NTB)]
            bias_tiles = {}

            def emit_bias_dma(gi):
                if gi >= len(groups):
                    return
                b, tb = groups[gi]
                bts = []
                for a in range(HPC):
                    t_ = bias_pool.tile([128, NST, T_BLOCK], BF16,
                                        tag=f"bias{a}", name=f"bias{b}_{tb}_{a}")
                    nc.gpsimd.dma_start(
                        out=t_[:],
                        in_=biasT_in[b, a, :, tb * T_BLOCK:(tb + 1) * T_BLOCK]
                        .rearrange("(st p) t -> p st t", p=128))
                    bts.append(t_)
                bias_tiles[gi] = bts

            emit_bias_dma(0)

            # ---------------- phase 1: projections ----------------
            with tc.tile_pool(name="hst", bufs=1) as hst_pool, \
                 tc.tile_pool(name="proj_ps", bufs=3, space="PSUM") as proj_ps:
                hstrips = {}
                for bb2 in range(B):
                    for e in range(NE):
                        h = hst_pool.tile([128, T], BF16, tag=f"hst{bb2}_{e}",
                                          name=f"hst{bb2}_{e}")
                        nc.sync.dma_start(
                            out=h[:], in_=hsT[e * 128:(e + 1) * 128,
                                              bb2 * T:(bb2 + 1) * T])
                        hstrips[(bb2, e)] = h
                for pb in range(NPB):
                    t0 = pb * PROJ_BLOCK
                    bb = t0 // T
                    tloc = t0 % T
                    for name, dstl in (("wq", QTb), ("wk", KTb), ("wv", VTb)):
                        ps = proj_ps.tile([128, PROJ_BLOCK], F32, tag="proj",
                                          name=f"pps{pb}_{name}")
                        for e in range(NE):
                            nc.tensor.matmul(ps[:], w_sb[name][:, e, :],
                                             hstrips[(bb, e)][:, tloc:tloc + PROJ_BLOCK],
                                             start=(e == 0), stop=(e == NE - 1))
                        nc.scalar.activation(
                            out=dstl[bb][:, tloc:tloc + PROJ_BLOCK], in_=ps[:],
                            func=mybir.ActivationFunctionType.Identity,
                            bias=b_sb["b" + name[1]][:], scale=1.0)

            # ---------------- phase 1b: V natural ----------------
            with tc.tile_pool(name="vtr_ps", bufs=2, space="PSUM") as vtr_ps:
                for bb in range(B):
                    for st in range(T // 128):
                        ps = vtr_ps.tile([128, 128], BF16, tag="vtr",
                                         name=f"vtr{bb}_{st}")
                        nc.tensor.transpose(ps[:], VTb[bb][:, st * 128:(st + 1) * 128],
                                            i_bf[:])
                        nc.vector.tensor_copy(out=V_sbb[bb][:, st, 0:D],
                                              in_=ps[:, 0:D])
                        nc.vector.tensor_copy(out=V_sbb[bb][:, st, 128:128 + D],
                                              in_=ps[:, D:2 * D])

            # ---------------- phase 2: attention ----------------
            with tc.tile_pool(name="stage", bufs=4) as stage_pool, \
                 tc.tile_pool(name="e_sb", bufs=6) as e_pool, \
                 tc.tile_pool(name="ot_sb", bufs=2) as ot_sb_pool, \
                 tc.tile_pool(name="rr", bufs=4) as r_pool, \
                 tc.tile_pool(name="rb", bufs=4) as rb_pool, \
                 tc.tile_pool(name="osb", bufs=3) as out_pool, \
                 tc.tile_pool(name="st_ps", bufs=2, space="PSUM") as st_ps, \
                 tc.tile_pool(name="ot_ps", bufs=2, space="PSUM") as ot_ps, \
                 tc.tile_pool(name="wo_ps", bufs=2, space="PSUM") as wo_ps:

                # out-projection of the PREVIOUS group, emitted one (k, half)
                # item per sp iteration so the wo PSUM ring never stalls PE
                pending_wo = []   # list of closures
                os_cur = {}

                def make_wo_items(otn_p, tglob_p):
                    items = []
                    for k in range(NJ):
                        for h2 in range(2):
                            def item(k=k, h2=h2, otn_p=otn_p, tglob_p=tglob_p):
                                if h2 == 0:
                                    os_cur['t'] = out_pool.tile(
                                        [128, E], BF16, tag="osb",
                                        name=f"osb{tglob_p}_{k}")
                                os_t = os_cur['t']
                                wp = wo_ps.tile([128, 512], F32, tag="wo",
                                                name=f"wop{tglob_p}_{k}_{h2}")
                                nc.tensor.matmul(
                                    wp[:], otn_p[:, k * 128:(k + 1) * 128],
                                    wo_sb[:, h2 * 512:(h2 + 1) * 512],
                                    start=True, stop=True)
                                if h2 == 0:
                                    nc.vector.tensor_copy(
                                        out=os_t[:, 0:512], in_=wp[:])
                                else:
                                    nc.scalar.copy(
                                        out=os_t[:, 512:1024], in_=wp[:])
                                if h2 == 1:
                                    nc.sync.dma_start(
                                        out=out_partial[tglob_p + k * 128:
                                                        tglob_p + (k + 1) * 128, :],
                                        in_=os_t[:])
                            items.append(item)
                    return items

                for gi, (b, tb) in enumerate(groups):
                    tglob = b * T + tb * T_BLOCK
                    emit_bias_dma(gi + 1)
                    bt = bias_tiles.pop(gi)

                    ots = [ot_ps.tile([128, T_BLOCK], F32, tag="ot",
                                      name=f"ot{b}_{tb}_{a}") for a in range(HPC)]

                    def emit_pv(pend):
                        for a, e_sl, pst in pend:
                            nc.tensor.matmul(
                                ots[a][:],
                                V_sbb[b][:, pst, a * 128:a * 128 + 128],
                                e_sl,
                                start=(pst == 0), stop=(pst == NST - 1))

                    pendq = []
                    for sp in range(NST // 2):
                        sts = [st_ps.tile([128, 2, T_BLOCK], F32, tag="st",
                                          name=f"st{b}_{tb}_{sp}_{a}")
                               for a in range(HPC)]
                        for a in range(HPC):
                            r0 = a * D
                            pe_unit = ASSIGN[sp * 2 + a] == 'p'
                            for half in range(2):
                                st_i = sp * 2 + half
                                nc.tensor.matmul(
                                    sts[a][:, half, :],
                                    KTb[b][r0:r0 + D, st_i * 128:(st_i + 1) * 128],
                                    QTb[b][r0:r0 + D,
                                           tb * T_BLOCK:(tb + 1) * T_BLOCK],
                                    start=True, stop=not pe_unit)
                        if len(pendq) >= 2:
                            emit_pv(pendq.pop(0))
                        if pending_wo:
                            pending_wo.pop(0)()
                        pend = []
                        for a in range(HPC):
                            pe_unit = ASSIGN[sp * 2 + a] == 'p'
                            e_t = e_pool.tile([128, 2, T_BLOCK], BF16, tag="et",
                                              name=f"et{b}_{tb}_{sp}_{a}")
                            if pe_unit:
                                for half in range(2):
                                    st_i = sp * 2 + half
                                    nc.tensor.matmul(
                                        sts[a][:, half, :], i_bf[:],
                                        bt[a][:, st_i, :],
                                        start=False, stop=True)
                                nc.scalar.activation(
                                    out=e_t[:], in_=sts[a][:],
                                    func=mybir.ActivationFunctionType.Exp)
                            else:
                                stg = stage_pool.tile([128, 2, T_BLOCK], F32,
                                                      tag="stg",
                                                      name=f"stg{b}_{tb}_{sp}_{a}")
                                nc.vector.tensor_add(
                                    out=stg[:], in0=sts[a][:],
                                    in1=bt[a][:, sp * 2:sp * 2 + 2, :])
                                nc.scalar.activation(
                                    out=e_t[:], in_=stg[:],
                                    func=mybir.ActivationFunctionType.Exp)
                            for half in range(2):
                                pend.append((a, e_t[:, half, :], sp * 2 + half))
                        pendq.append(pend)
                    for pend in pendq:
                        emit_pv(pend)
                    while pending_wo:
                        pending_wo.pop(0)()

                    # per head: r = 1/sums row; broadcast to D partitions;
                    # drain O.T fused with the r scaling
                    otn = ot_sb_pool.tile([128, T_BLOCK], BF16, tag="otn",
                                          name=f"otn{b}_{tb}")
                    for a in range(HPC):
                        rr = r_pool.tile([1, T_BLOCK], F32, tag="rr",
                                         name=f"rr{b}_{tb}_{a}")
                        nc.vector.reciprocal(rr[:], ots[a][D:D + 1, :])
                        rb = rb_pool.tile([D, T_BLOCK], F32, tag="rb",
                                          name=f"rb{b}_{tb}_{a}")
                        idx = gi * HPC + a
                        nc.sync.dma_start(out=rc_dram[idx, :], in_=rr[:])
                        src = bass.AP(rc_dram[:].tensor, idx * T_BLOCK,
                                      [[0, D], [1, T_BLOCK]])
                        nc.sync.dma_start(out=rb[:], in_=src)
                        nc.vector.tensor_mul(out=otn[a * D:(a + 1) * D, :],
                                             in0=ots[a][0:D, :], in1=rb[:])
                    pending_wo = make_wo_items(otn, tglob)
                while pending_wo:
                    pending_wo.pop(0)()
    _waitfix(nc)
    return nc


# ---------------- host-side prep ----------------

def shard_inputs(hidden_states, attn_bias, attention_mask, Wq, bq, Wk, bk, Wv, bv,
                 Wo, bo, c_attn, n_cores=8, scaling=None):
    """Build per-core input maps. Returns (in_maps, with_mask=False); the
    attention mask (when nonzero) is folded into the bias on the host."""
    import ml_dtypes
    bf16 = ml_dtypes.bfloat16
    B, T, E = hidden_states.shape
    NH = c_attn.shape[0]
    D = E // NH
    HPC = NH // n_cores
    DH = HPC * D

    hsT = np.ascontiguousarray(hidden_states.reshape(B * T, E).T).astype(bf16)
    bias4 = attn_bias.reshape(B, NH, T, T)
    if np.any(attention_mask):
        bias4 = bias4 + attention_mask.reshape(B, 1, T, T)

    if scaling is None:
        scaling = float(D * 2.0) ** -0.5

    in_maps = []
    for c in range(n_cores):
        r0 = c * DH
        sl = slice(r0, r0 + DH)
        cvec = np.repeat(c_attn[c * HPC:(c + 1) * HPC], D)
        biasTc = np.ascontiguousarray(
            bias4[:, c * HPC:(c + 1) * HPC].transpose(0, 1, 3, 2)).astype(bf16)
        m = {
            "hsT": hsT,
            "wqT": np.ascontiguousarray((Wq[sl] * scaling).T).astype(bf16),
            "wkT": np.ascontiguousarray(Wk[sl].T).astype(bf16),
            "wvT": np.ascontiguousarray((Wv[sl] * cvec[:, None]).T).astype(bf16),
            "bq": np.ascontiguousarray((bq[sl] * scaling)[:, None]).astype(np.float32),
            "bk": np.ascontiguousarray(bk[sl][:, None]).astype(np.float32),
            "bv": np.ascontiguousarray((bv[sl] * cvec)[:, None]).astype(np.float32),
            "woT": np.ascontiguousarray(Wo[:, sl].T).astype(bf16),
            "biasT": biasTc,
        }
        in_maps.append(m)
    return in_maps, False


_NC_CACHE = {}


def run_spmd(in_maps, with_mask=False, **kwargs):
    if "v2" not in _NC_CACHE:
        _NC_CACHE["v2"] = build_attention_nc(B=B, T=T, E=E, HPC=HPC, D=D)
    nc = _NC_CACHE["v2"]
    return run_bass_kernel_spmd(nc, in_maps, list(range(N_CORES)), **kwargs)


def kernel(hidden_states, attn_bias, attention_mask, Wq, bq, Wk, bk, Wv, bv,
           Wo, bo, c_attn):
    args = [np.asarray(a, dtype=np.float32) for a in
            (hidden_states, attn_bias, attention_mask, Wq, bq, Wk, bk, Wv, bv,
             Wo, bo, c_attn)]
    (hidden_states, attn_bias, attention_mask, Wq, bq, Wk, bk, Wv, bv,
     Wo, bo, c_attn) = args
    in_maps, with_mask = shard_inputs(hidden_states, attn_bias, attention_mask,
                                      Wq, bq, Wk, bk, Wv, bv, Wo, bo, c_attn,
                                      n_cores=N_CORES, scaling=SCALING)
    res = run_spmd(in_maps, with_mask)
    out = np.zeros((B * T, E), np.float32)
    for r in res.results:
        out += r["out"]
    out += bo[None, :]
    return out.reshape(B, T, E).astype(np.float32)


# revision 14
# speedup vs baseline: 1.1044x; 1.1044x over previous
"""OFA attention (dense_transformer) on 8 Trainium2 NeuronCores.

Sharding: heads split over cores (core c owns heads {2c, 2c+1}, both batches).
Per-core Bass/Tile program (build_attention_nc):
  phase 1 : QT/KT/VT = W_c @ hs.T (transposed projections; SCALING folded into Wq,
            c_attn folded into Wv on host; bias-add fused into PSUM drain on ScalarE)
  phase 1b: V natural = PE-transpose(VT), packed [V_A | 1 | V_B | 1] bf16
  phase 2 : per (batch, 512-token t-block), streaming 128-row s-tiles:
              ST(s,t) = K Q^T            (row-tiled K=64 matmuls)
              ST += bias.T               (bias pre-transposed to [s,t] bf16 on HOST;
                                          adds split: some s-tiles accumulate via a
                                          resident-identity matmul on PE, the rest on
                                          DVE tensor_add into an f32 staging tile)
              E = exp(ST)                (ScalarE only; exp without max-subtraction:
                                          scores+bias stay in [-8, 8])
              [O.T ; sums] += [V|1].T@E  (PV matmul also produces softmax denominators)
            per head: r = 1/sums (DVE), partition_broadcast (GpSimd), O.T drain fused
            with the r scaling (DVE tensor_mul) -> out-projection runs at full
            contract-128 (both heads in one matmul), spread across the next group's
            sp iterations; drains split DVE/GpSimd.
Host: partial outputs summed over cores + bo (the "all-reduce" of the out-projection).
"""
import sys

for _p in ("/opt/trn_rl_repo",):
    if _p not in sys.path:
        sys.path.append(_p)

import numpy as np

import concourse.bass as bass
import concourse.tile as tile
from concourse import mybir
from concourse.masks import make_identity
from concourse.bass_utils import run_bass_kernel_spmd

F32 = mybir.dt.float32
BF16 = mybir.dt.bfloat16

B, T, E, NH, D = 2, 2048, 1024, 16, 64
N_CORES = 8
HPC = NH // N_CORES
DH = HPC * D
SCALING = float(D * 2.0) ** -0.5

# Bias-add engine per unit u = sp*2 + a: 'p' = PE identity-matmul, 'v' = DVE.
# (GpSimd cannot read PSUM, so it only does broadcasts + bias DMA triggers.)
ASSIGN = ['p', 'p', 'v', 'v', 'p', 'v', 'v', 'p',
          'v', 'v', 'p', 'v', 'v', 'p', 'v', 'v']


def _waitfix(nc, limit=1):
    """This walrus build accepts at most ONE sync-wait per instruction.
    Hoist excess sem-waits onto inserted single-wait NoOps."""
    n_fixed = 0
    for bb in nc.m.functions[0].blocks:
        i = 0
        insts = bb.instructions
        while i < len(insts):
            inst = insts[i]
            si = inst.sync_info
            if si and si.on_wait and len(si.on_wait) > limit:
                extra = si.on_wait[limit:]
                si.on_wait = si.on_wait[:limit]
                for k, w in enumerate(extra):
                    nop = mybir.InstNoOp(
                        name=f"{inst.name}-waitfix{k}",
                        engine=inst.engine,
                        sync_info=mybir.SyncInfo(on_wait=[w], on_update=[]),
                        bass_nofuse=True,
                    )
                    nc.register_instruction(nop, overwrite=True)
                    insts.insert(i, nop)
                    i += 1
                n_fixed += 1
            i += 1
    return n_fixed


def build_attention_nc(B=2, T=2048, E=1024, HPC=2, D=64,
                       T_BLOCK=512, PROJ_BLOCK=512):
    """Build the per-core Bass program. Returns nc."""
    S = T
    PROJ_BLOCK = min(PROJ_BLOCK, T)
    TOK = B * T
    DH = HPC * D                      # 128
    assert DH == 128 and D == 64 and HPC == 2
    NE = E // 128                     # e-tiles
    NST = S // 128                    # s-tiles per batch
    NTB = T // T_BLOCK                # t-blocks per batch
    NJ = T_BLOCK // 128               # t-subtiles per block
    NPB = TOK // PROJ_BLOCK           # proj token blocks

    nc = bass.Bass()

    hsT = nc.declare_dram_parameter("hsT", [E, TOK], BF16, isOutput=False)
    wqT = nc.declare_dram_parameter("wqT", [E, DH], BF16, isOutput=False)
    wkT = nc.declare_dram_parameter("wkT", [E, DH], BF16, isOutput=False)
    wvT = nc.declare_dram_parameter("wvT", [E, DH], BF16, isOutput=False)
    bq = nc.declare_dram_parameter("bq", [DH, 1], F32, isOutput=False)
    bk = nc.declare_dram_parameter("bk", [DH, 1], F32, isOutput=False)
    bv = nc.declare_dram_parameter("bv", [DH, 1], F32, isOutput=False)
    woT = nc.declare_dram_parameter("woT", [DH, E], BF16, isOutput=False)
    biasT_in = nc.declare_dram_parameter("biasT", [B, HPC, S, T], BF16,
                                         isOutput=False)
    out_partial = nc.declare_dram_parameter("out", [TOK, E], BF16, isOutput=True)
    rc_dram = nc.dram_tensor("rc_scratch", [B * NTB * HPC, T_BLOCK], F32)

    with tile.TileContext(nc) as tc:
        from contextlib import ExitStack
        with ExitStack() as ctx:
            consts = ctx.enter_context(tc.tile_pool(name="consts", bufs=1))
            persist = ctx.enter_context(tc.tile_pool(name="persist", bufs=1))
            bias_pool = ctx.enter_context(tc.tile_pool(name="bias_sb", bufs=2))

            i_bf = consts.tile([128, 128], BF16, tag="i_bf")
            make_identity(nc, i_bf[:])

            # weights: (E, DH) -> (128, NE, DH), bf16
            w_sb = {}
            for name, src in (("wq", wqT), ("wk", wkT), ("wv", wvT)):
                t = consts.tile([128, NE, DH], BF16, tag=name)
                nc.gpsimd.dma_start(out=t[:], in_=src.rearrange("(n p) d -> p n d", p=128))
                w_sb[name] = t
            wo_sb = consts.tile([128, E], BF16, tag="wo")
            nc.gpsimd.dma_start(out=wo_sb[:], in_=woT[:, :])
            b_sb = {}
            for name, src in (("bq", bq), ("bk", bk), ("bv", bv)):
                t = consts.tile([128, 1], F32, tag=name)
                nc.gpsimd.dma_start(out=t[:], in_=src[:, :])
                b_sb[name] = t

            # persistent activations
            QTb = [persist.tile([128, T], BF16, tag=f"QT{bb}", name=f"QT{bb}")
                   for bb in range(B)]
            KTb = [persist.tile([128, T], BF16, tag=f"KT{bb}", name=f"KT{bb}")
                   for bb in range(B)]
            VTb = [persist.tile([128, T], BF16, tag=f"VT{bb}", name=f"VT{bb}")
                   for bb in range(B)]
            V_sbb = []
            for bb in range(B):
                V_sb = persist.tile([128, T // 128, 256], BF16, tag=f"V_sb{bb}",
                                    name=f"V_sb{bb}")
                nc.vector.memset(V_sb[:, :, :], 0.0)
                nc.vector.memset(V_sb[:, :, D:D + 1], 1.0)
                nc.vector.memset(V_sb[:, :, 128 + D:128 + D + 1], 1.0)
                V_sbb.append(V_sb)

            # group list + bias prefetch plumbing (1-group emission lookahead;
            # DMAs ride the otherwise-idle GpSimd queue, transfers overlap the
            # previous group / phase 1)
            groups = [(b, tb) for b in range(B) for tb in range(NTB)]
            bias_tiles = {}

            def emit_bias_dma(gi):
                if gi >= len(groups):
                    return
                b, tb = groups[gi]
                bts = []
                for a in range(HPC):
                    t_ = bias_pool.tile([128, NST, T_BLOCK], BF16,
                                        tag=f"bias{a}", name=f"bias{b}_{tb}_{a}")
                    nc.gpsimd.dma_start(
                        out=t_[:],
                        in_=biasT_in[b, a, :, tb * T_BLOCK:(tb + 1) * T_BLOCK]
                        .rearrange("(st p) t -> p st t", p=128))
                    bts.append(t_)
                bias_tiles[gi] = bts

            # ---------------- phase 1: projections ----------------
            # all input loads ride the gpsimd queue IN ORDER (weights, then
            # hsT strips, then bias prefetch) so bias cannot starve phase 1
            with tc.tile_pool(name="hst", bufs=1) as hst_pool, \
                 tc.tile_pool(name="proj_ps", bufs=3, space="PSUM") as proj_ps:
                hstrips = {}
                for bb2 in range(B):
                    for e in range(NE):
                        h = hst_pool.tile([128, T], BF16, tag=f"hst{bb2}_{e}",
                                          name=f"hst{bb2}_{e}")
                        nc.gpsimd.dma_start(
                            out=h[:], in_=hsT[e * 128:(e + 1) * 128,
                                              bb2 * T:(bb2 + 1) * T])
                        hstrips[(bb2, e)] = h
                emit_bias_dma(0)
                emit_bias_dma(1)
                for pb in range(NPB):
                    t0 = pb * PROJ_BLOCK
                    bb = t0 // T
                    tloc = t0 % T
                    for name, dstl in (("wq", QTb), ("wk", KTb), ("wv", VTb)):
                        ps = proj_ps.tile([128, PROJ_BLOCK], F32, tag="proj",
                                          name=f"pps{pb}_{name}")
                        for e in range(NE):
                            nc.tensor.matmul(ps[:], w_sb[name][:, e, :],
                                             hstrips[(bb, e)][:, tloc:tloc + PROJ_BLOCK],
                                             start=(e == 0), stop=(e == NE - 1))
                        nc.scalar.activation(
                            out=dstl[bb][:, tloc:tloc + PROJ_BLOCK], in_=ps[:],
                            func=mybir.ActivationFunctionType.Identity,
                            bias=b_sb["b" + name[1]][:], scale=1.0)

            # ---------------- phase 1b: V natural ----------------
            with tc.tile_pool(name="vtr_ps", bufs=2, space="PSUM") as vtr_ps:
                for bb in range(B):
                    for st in range(T // 128):
                        ps = vtr_ps.tile([128, 128], BF16, tag="vtr",
                                         name=f"vtr{bb}_{st}")
                        nc.tensor.transpose(ps[:], VTb[bb][:, st * 128:(st + 1) * 128],
                                            i_bf[:])
                        nc.vector.tensor_copy(out=V_sbb[bb][:, st, 0:D],
                                              in_=ps[:, 0:D])
                        nc.vector.tensor_copy(out=V_sbb[bb][:, st, 128:128 + D],
                                              in_=ps[:, D:2 * D])

            # ---------------- phase 2: attention ----------------
            with tc.tile_pool(name="stage", bufs=4) as stage_pool, \
                 tc.tile_pool(name="e_sb", bufs=6) as e_pool, \
                 tc.tile_pool(name="ot_sb", bufs=2) as ot_sb_pool, \
                 tc.tile_pool(name="rr", bufs=4) as r_pool, \
                 tc.tile_pool(name="rb", bufs=4) as rb_pool, \
                 tc.tile_pool(name="osb", bufs=3) as out_pool, \
                 tc.tile_pool(name="st_ps", bufs=2, space="PSUM") as st_ps, \
                 tc.tile_pool(name="ot_ps", bufs=2, space="PSUM") as ot_ps, \
                 tc.tile_pool(name="wo_ps", bufs=2, space="PSUM") as wo_ps:

                # out-projection of the PREVIOUS group, emitted one (k, half)
                # item per sp iteration so the wo PSUM ring never stalls PE
                pending_wo = []   # list of closures
                os_cur = {}

                def make_wo_items(otn_p, tglob_p):
                    items = []
                    for k in range(NJ):
                        for h2 in range(2):
                            def item(k=k, h2=h2, otn_p=otn_p, tglob_p=tglob_p):
                                if h2 == 0:
                                    os_cur['t'] = out_pool.tile(
                                        [128, E], BF16, tag="osb",
                                        name=f"osb{tglob_p}_{k}")
                                os_t = os_cur['t']
                                wp = wo_ps.tile([128, 512], F32, tag="wo",
                                                name=f"wop{tglob_p}_{k}_{h2}")
                                nc.tensor.matmul(
                                    wp[:], otn_p[:, k * 128:(k + 1) * 128],
                                    wo_sb[:, h2 * 512:(h2 + 1) * 512],
                                    start=True, stop=True)
                                if h2 == 0:
                                    nc.vector.tensor_copy(
                                        out=os_t[:, 0:512], in_=wp[:])
                                else:
                                    nc.scalar.copy(
                                        out=os_t[:, 512:1024], in_=wp[:])
                                if h2 == 1:
                                    nc.sync.dma_start(
                                        out=out_partial[tglob_p + k * 128:
                                                        tglob_p + (k + 1) * 128, :],
                                        in_=os_t[:])
                            items.append(item)
                    return items

                for gi, (b, tb) in enumerate(groups):
                    tglob = b * T + tb * T_BLOCK
                    emit_bias_dma(gi + 2)
                    bt = bias_tiles.pop(gi)

                    ots = [ot_ps.tile([128, T_BLOCK], F32, tag="ot",
                                      name=f"ot{b}_{tb}_{a}") for a in range(HPC)]

                    def emit_pv(pend):
                        for a, e_sl, pst in pend:
                            nc.tensor.matmul(
                                ots[a][:],
                                V_sbb[b][:, pst, a * 128:a * 128 + 128],
                                e_sl,
                                start=(pst == 0), stop=(pst == NST - 1))

                    pendq = []
                    for sp in range(NST // 2):
                        sts = [st_ps.tile([128, 2, T_BLOCK], F32, tag="st",
                                          name=f"st{b}_{tb}_{sp}_{a}")
                               for a in range(HPC)]
                        for a in range(HPC):
                            r0 = a * D
                            pe_unit = ASSIGN[sp * 2 + a] == 'p'
                            for half in range(2):
                                st_i = sp * 2 + half
                                nc.tensor.matmul(
                                    sts[a][:, half, :],
                                    KTb[b][r0:r0 + D, st_i * 128:(st_i + 1) * 128],
                                    QTb[b][r0:r0 + D,
                                           tb * T_BLOCK:(tb + 1) * T_BLOCK],
                                    start=True, stop=not pe_unit)
                        if len(pendq) >= 2:
                            emit_pv(pendq.pop(0))
                        if pending_wo and sp >= 2:
                            pending_wo.pop(0)()
                        pend = []
                        for a in range(HPC):
                            pe_unit = ASSIGN[sp * 2 + a] == 'p'
                            e_t = e_pool.tile([128, 2, T_BLOCK], BF16, tag="et",
                                              name=f"et{b}_{tb}_{sp}_{a}")
                            if pe_unit:
                                for half in range(2):
                                    st_i = sp * 2 + half
                                    nc.tensor.matmul(
                                        sts[a][:, half, :], i_bf[:],
                                        bt[a][:, st_i, :],
                                        start=False, stop=True)
                                nc.scalar.activation(
                                    out=e_t[:], in_=sts[a][:],
                                    func=mybir.ActivationFunctionType.Exp)
                            else:
                                stg = stage_pool.tile([128, 2, T_BLOCK], F32,
                                                      tag="stg",
                                                      name=f"stg{b}_{tb}_{sp}_{a}")
                                nc.vector.tensor_add(
                                    out=stg[:], in0=sts[a][:],
                                    in1=bt[a][:, sp * 2:sp * 2 + 2, :])
                                nc.scalar.activation(
                                    out=e_t[:], in_=stg[:],
                                    func=mybir.ActivationFunctionType.Exp)
                            for half in range(2):
                                pend.append((a, e_t[:, half, :], sp * 2 + half))
                        pendq.append(pend)
                    for pend in pendq:
                        emit_pv(pend)
                    while pending_wo:
                        pending_wo.pop(0)()

                    # per head: r = 1/sums row; broadcast to D partitions;
                    # drain O.T fused with the r scaling
                    otn = ot_sb_pool.tile([128, T_BLOCK], BF16, tag="otn",
                                          name=f"otn{b}_{tb}")
                    for a in range(HPC):
                        # 1/sums = exp(-ln(sums)) on ScalarE: DVE reciprocal on a
                        # [1,512] row costs 3.3us and head-of-line-blocks DVE
                        ls = r_pool.tile([1, T_BLOCK], F32, tag="ls",
                                         name=f"ls{b}_{tb}_{a}")
                        nc.scalar.activation(out=ls[:], in_=ots[a][D:D + 1, :],
                                             func=mybir.ActivationFunctionType.Ln)
                        rr = r_pool.tile([1, T_BLOCK], F32, tag="rr",
                                         name=f"rr{b}_{tb}_{a}")
                        nc.scalar.activation(out=rr[:], in_=ls[:],
                                             func=mybir.ActivationFunctionType.Exp,
                                             scale=-1.0)
                        rb = rb_pool.tile([D, T_BLOCK], F32, tag="rb",
                                          name=f"rb{b}_{tb}_{a}")
                        idx = gi * HPC + a
                        nc.sync.dma_start(out=rc_dram[idx, :], in_=rr[:])
                        src = bass.AP(rc_dram[:].tensor, idx * T_BLOCK,
                                      [[0, D], [1, T_BLOCK]])
                        nc.sync.dma_start(out=rb[:], in_=src)
                        nc.vector.tensor_mul(out=otn[a * D:(a + 1) * D, :],
                                             in0=ots[a][0:D, :], in1=rb[:])
                    pending_wo = make_wo_items(otn, tglob)
                while pending_wo:
                    pending_wo.pop(0)()
    _waitfix(nc)
    return nc


# ---------------- host-side prep ----------------

def shard_inputs(hidden_states, attn_bias, attention_mask, Wq, bq, Wk, bk, Wv, bv,
                 Wo, bo, c_attn, n_cores=8, scaling=None):
    """Build per-core input maps. Returns (in_maps, with_mask=False); the
    attention mask (when nonzero) is folded into the bias on the host."""
    import ml_dtypes
    bf16 = ml_dtypes.bfloat16
    B, T, E = hidden_states.shape
    NH = c_attn.shape[0]
    D = E // NH
    HPC = NH // n_cores
    DH = HPC * D

    hsT = np.ascontiguousarray(hidden_states.reshape(B * T, E).T).astype(bf16)
    bias4 = attn_bias.reshape(B, NH, T, T)
    if np.any(attention_mask):
        bias4 = bias4 + attention_mask.reshape(B, 1, T, T)

    if scaling is None:
        scaling = float(D * 2.0) ** -0.5

    in_maps = []
    for c in range(n_cores):
        r0 = c * DH
        sl = slice(r0, r0 + DH)
        cvec = np.repeat(c_attn[c * HPC:(c + 1) * HPC], D)
        biasTc = np.ascontiguousarray(
            bias4[:, c * HPC:(c + 1) * HPC].transpose(0, 1, 3, 2)).astype(bf16)
        m = {
            "hsT": hsT,
            "wqT": np.ascontiguousarray((Wq[sl] * scaling).T).astype(bf16),
            "wkT": np.ascontiguousarray(Wk[sl].T).astype(bf16),
            "wvT": np.ascontiguousarray((Wv[sl] * cvec[:, None]).T).astype(bf16),
            "bq": np.ascontiguousarray((bq[sl] * scaling)[:, None]).astype(np.float32),
            "bk": np.ascontiguousarray(bk[sl][:, None]).astype(np.float32),
            "bv": np.ascontiguousarray((bv[sl] * cvec)[:, None]).astype(np.float32),
            "woT": np.ascontiguousarray(Wo[:, sl].T).astype(bf16),
            "biasT": biasTc,
        }
        in_maps.append(m)
    return in_maps, False


_NC_CACHE = {}


def run_spmd(in_maps, with_mask=False, **kwargs):
    if "v2" not in _NC_CACHE:
        _NC_CACHE["v2"] = build_attention_nc(B=B, T=T, E=E, HPC=HPC, D=D)
    nc = _NC_CACHE["v2"]
    return run_bass_kernel_spmd(nc, in_maps, list(range(N_CORES)), **kwargs)


def kernel(hidden_states, attn_bias, attention_mask, Wq, bq, Wk, bk, Wv, bv,
           Wo, bo, c_attn):
    args = [np.asarray(a, dtype=np.float32) for a in
            (hidden_states, attn_bias, attention_mask, Wq, bq, Wk, bk, Wv, bv,
             Wo, bo, c_attn)]
    (hidden_states, attn_bias, attention_mask, Wq, bq, Wk, bk, Wv, bv,
     Wo, bo, c_attn) = args
    in_maps, with_mask = shard_inputs(hidden_states, attn_bias, attention_mask,
                                      Wq, bq, Wk, bk, Wv, bv, Wo, bo, c_attn,
                                      n_cores=N_CORES, scaling=SCALING)
    res = run_spmd(in_maps, with_mask)
    out = np.zeros((B * T, E), np.float32)
    for r in res.results:
        out += r["out"]
    out += bo[None, :]
    return out.reshape(B, T, E).astype(np.float32)


# revision 21
# speedup vs baseline: 1.1794x; 1.0680x over previous
"""OFA attention (dense_transformer) on 8 Trainium2 NeuronCores.

Sharding: heads split over cores (core c owns heads {2c, 2c+1}, both batches).
Per-core Bass/Tile program (build_attention_nc):
  phase 1 : QT/KT/VT = W_c @ hs.T (transposed projections; SCALING folded into Wq,
            c_attn folded into Wv on host; bias-add fused into PSUM drain on ScalarE)
  phase 1b: V natural = PE-transpose(VT), packed [V_A | 1 | V_B | 1] bf16
  phase 2 : per (batch, 512-token t-block), streaming 128-row s-tiles:
              ST(s,t) = K Q^T            (row-tiled K=64 matmuls)
              ST += bias.T               (bias pre-transposed to [s,t] bf16 on HOST;
                                          adds split: some s-tiles accumulate via a
                                          resident-identity matmul on PE, the rest on
                                          DVE tensor_add into an f32 staging tile)
              E = exp(ST)                (ScalarE only; exp without max-subtraction:
                                          scores+bias stay in [-8, 8])
              [O.T ; sums] += [V|1].T@E  (PV matmul also produces softmax denominators)
            per head: r = 1/sums (DVE), partition_broadcast (GpSimd), O.T drain fused
            with the r scaling (DVE tensor_mul) -> out-projection runs at full
            contract-128 (both heads in one matmul), spread across the next group's
            sp iterations; drains split DVE/GpSimd.
Host: partial outputs summed over cores + bo (the "all-reduce" of the out-projection).
"""
import sys

for _p in ("/opt/trn_rl_repo",):
    if _p not in sys.path:
        sys.path.append(_p)

import numpy as np

import concourse.bass as bass
import concourse.tile as tile
from concourse import mybir
from concourse.masks import make_identity
from concourse.bass_utils import run_bass_kernel_spmd

F32 = mybir.dt.float32
BF16 = mybir.dt.bfloat16

B, T, E, NH, D = 2, 2048, 1024, 16, 64
N_CORES = 8
HPC = NH // N_CORES
DH = HPC * D
SCALING = float(D * 2.0) ** -0.5

# Bias-add engine per unit u = sp*2 + a: 'p' = PE identity-matmul, 'v' = DVE.
# (GpSimd cannot read PSUM, so it only does broadcasts + bias DMA triggers.)
ASSIGN = ['p', 'p', 'v', 'v', 'p', 'v', 'p', 'v',
          'v', 'p', 'v', 'p', 'v', 'p', 'v', 'p']


def _waitfix(nc, limit=1):
    """This walrus build accepts at most ONE sync-wait per instruction.
    Hoist excess sem-waits onto inserted single-wait NoOps."""
    n_fixed = 0
    for bb in nc.m.functions[0].blocks:
        i = 0
        insts = bb.instructions
        while i < len(insts):
            inst = insts[i]
            si = inst.sync_info
            if si and si.on_wait and len(si.on_wait) > limit:
                extra = si.on_wait[limit:]
                si.on_wait = si.on_wait[:limit]
                for k, w in enumerate(extra):
                    nop = mybir.InstNoOp(
                        name=f"{inst.name}-waitfix{k}",
                        engine=inst.engine,
                        sync_info=mybir.SyncInfo(on_wait=[w], on_update=[]),
                        bass_nofuse=True,
                    )
                    nc.register_instruction(nop, overwrite=True)
                    insts.insert(i, nop)
                    i += 1
                n_fixed += 1
            i += 1
    return n_fixed


def build_attention_nc(B=2, T=2048, E=1024, HPC=2, D=64,
                       T_BLOCK=512, PROJ_BLOCK=512):
    """Build the per-core Bass program. Returns nc."""
    S = T
    PROJ_BLOCK = min(PROJ_BLOCK, T)
    TOK = B * T
    DH = HPC * D                      # 128
    assert DH == 128 and D == 64 and HPC == 2
    NE = E // 128                     # e-tiles
    NST = S // 128                    # s-tiles per batch
    NTB = T // T_BLOCK                # t-blocks per batch
    NJ = T_BLOCK // 128               # t-subtiles per block
    NPB = TOK // PROJ_BLOCK           # proj token blocks

    nc = bass.Bass()

    hsT = nc.declare_dram_parameter("hsT", [E, TOK], BF16, isOutput=False)
    wqT = nc.declare_dram_parameter("wqT", [E, DH], BF16, isOutput=False)
    wkT = nc.declare_dram_parameter("wkT", [E, DH], BF16, isOutput=False)
    wvT = nc.declare_dram_parameter("wvT", [E, DH], BF16, isOutput=False)
    bq = nc.declare_dram_parameter("bq", [DH, 1], F32, isOutput=False)
    bk = nc.declare_dram_parameter("bk", [DH, 1], F32, isOutput=False)
    bv = nc.declare_dram_parameter("bv", [DH, 1], F32, isOutput=False)
    woT = nc.declare_dram_parameter("woT", [DH, E], BF16, isOutput=False)
    biasT_in = nc.declare_dram_parameter(
        "biasT", [B, HPC, T // T_BLOCK, 128, S // 128, T_BLOCK], BF16,
        isOutput=False)
    out_partial = nc.declare_dram_parameter("out", [TOK, E], BF16, isOutput=True)
    rc_dram = nc.dram_tensor("rc_scratch", [B * NTB * HPC, T_BLOCK], F32)

    with tile.TileContext(nc) as tc:
        from contextlib import ExitStack
        with ExitStack() as ctx:
            consts = ctx.enter_context(tc.tile_pool(name="consts", bufs=1))
            persist = ctx.enter_context(tc.tile_pool(name="persist", bufs=1))
            bias_pool = ctx.enter_context(tc.tile_pool(name="bias_sb", bufs=2))

            i_bf = consts.tile([128, 128], BF16, tag="i_bf")
            make_identity(nc, i_bf[:])

            # weights: (E, DH) -> (128, NE, DH), bf16
            w_sb = {}
            for name, src in (("wq", wqT), ("wk", wkT), ("wv", wvT)):
                t = consts.tile([128, NE, DH], BF16, tag=name)
                nc.gpsimd.dma_start(out=t[:], in_=src.rearrange("(n p) d -> p n d", p=128))
                w_sb[name] = t
            wo_sb = consts.tile([128, E], BF16, tag="wo")
            nc.gpsimd.dma_start(out=wo_sb[:], in_=woT[:, :])
            b_sb = {}
            for name, src in (("bq", bq), ("bk", bk), ("bv", bv)):
                t = consts.tile([128, 1], F32, tag=name)
                nc.gpsimd.dma_start(out=t[:], in_=src[:, :])
                b_sb[name] = t

            # persistent activations
            QTb = [persist.tile([128, T], BF16, tag=f"QT{bb}", name=f"QT{bb}")
                   for bb in range(B)]
            KTb = [persist.tile([128, T], BF16, tag=f"KT{bb}", name=f"KT{bb}")
                   for bb in range(B)]
            VTb = [persist.tile([128, T], BF16, tag=f"VT{bb}", name=f"VT{bb}")
                   for bb in range(B)]
            V_sbb = []
            for bb in range(B):
                V_sb = persist.tile([128, T // 128, 256], BF16, tag=f"V_sb{bb}",
                                    name=f"V_sb{bb}")
                nc.vector.memset(V_sb[:, :, :], 0.0)
                nc.vector.memset(V_sb[:, :, D:D + 1], 1.0)
                nc.vector.memset(V_sb[:, :, 128 + D:128 + D + 1], 1.0)
                V_sbb.append(V_sb)

            # group list + bias prefetch plumbing (1-group emission lookahead;
            # DMAs ride the otherwise-idle GpSimd queue, transfers overlap the
            # previous group / phase 1)
            groups = [(b, tb) for b in range(B) for tb in range(NTB)]
            bias_tiles = {}

            def emit_bias_dma(gi):
                if gi >= len(groups):
                    return
                b, tb = groups[gi]
                bts = []
                for a in range(HPC):
                    t_ = bias_pool.tile([128, NST, T_BLOCK], BF16,
                                        tag=f"bias{a}", name=f"bias{b}_{tb}_{a}")
                    nc.gpsimd.dma_start(out=t_[:], in_=biasT_in[b, a, tb])
                    bts.append(t_)
                bias_tiles[gi] = bts

            # ---------------- phase 1: projections ----------------
            # all input loads ride the gpsimd queue IN ORDER (weights, then
            # hsT strips, then bias prefetch) so bias cannot starve phase 1
            with tc.tile_pool(name="hst", bufs=1) as hst_pool, \
                 tc.tile_pool(name="proj_ps", bufs=3, space="PSUM") as proj_ps:
                hstrips = {}
                for bb2 in range(B):
                    for q in range(T // 512):
                        for e in range(NE):
                            if q == 0:
                                h = hst_pool.tile([128, T], BF16,
                                                  tag=f"hst{bb2}_{e}",
                                                  name=f"hst{bb2}_{e}")
                                hstrips[(bb2, e)] = h
                            h = hstrips[(bb2, e)]
                            nc.gpsimd.dma_start(
                                out=h[:, q * 512:(q + 1) * 512],
                                in_=hsT[e * 128:(e + 1) * 128,
                                        bb2 * T + q * 512:bb2 * T + (q + 1) * 512])
                emit_bias_dma(0)
                emit_bias_dma(1)
                for pb in range(NPB):
                    t0 = pb * PROJ_BLOCK
                    bb = t0 // T
                    tloc = t0 % T
                    for name, dstl in (("wq", QTb), ("wk", KTb), ("wv", VTb)):
                        ps = proj_ps.tile([128, PROJ_BLOCK], F32, tag="proj",
                                          name=f"pps{pb}_{name}")
                        for e in range(NE):
                            nc.tensor.matmul(ps[:], w_sb[name][:, e, :],
                                             hstrips[(bb, e)][:, tloc:tloc + PROJ_BLOCK],
                                             start=(e == 0), stop=(e == NE - 1))
                        nc.scalar.activation(
                            out=dstl[bb][:, tloc:tloc + PROJ_BLOCK], in_=ps[:],
                            func=mybir.ActivationFunctionType.Identity,
                            bias=b_sb["b" + name[1]][:], scale=1.0)

            # ---------------- phase 1b: V natural ----------------
            with tc.tile_pool(name="vtr_ps", bufs=2, space="PSUM") as vtr_ps:
                for bb in range(B):
                    for st in range(T // 128):
                        ps = vtr_ps.tile([128, 128], BF16, tag="vtr",
                                         name=f"vtr{bb}_{st}")
                        nc.tensor.transpose(ps[:], VTb[bb][:, st * 128:(st + 1) * 128],
                                            i_bf[:])
                        nc.vector.tensor_copy(out=V_sbb[bb][:, st, 0:D],
                                              in_=ps[:, 0:D])
                        nc.vector.tensor_copy(out=V_sbb[bb][:, st, 128:128 + D],
                                              in_=ps[:, D:2 * D])

            # ---------------- phase 2: attention ----------------
            with tc.tile_pool(name="stage", bufs=4) as stage_pool, \
                 tc.tile_pool(name="e_sb", bufs=6) as e_pool, \
                 tc.tile_pool(name="ot_sb", bufs=2) as ot_sb_pool, \
                 tc.tile_pool(name="rr", bufs=4) as r_pool, \
                 tc.tile_pool(name="rb", bufs=4) as rb_pool, \
                 tc.tile_pool(name="osb", bufs=3) as out_pool, \
                 tc.tile_pool(name="st_ps", bufs=2, space="PSUM") as st_ps, \
                 tc.tile_pool(name="ot_ps", bufs=2, space="PSUM") as ot_ps, \
                 tc.tile_pool(name="wo_ps", bufs=2, space="PSUM") as wo_ps:

                # out-projection of the PREVIOUS group, emitted one (k, half)
                # item per sp iteration so the wo PSUM ring never stalls PE
                pending_wo = []   # list of closures
                os_cur = {}

                def make_wo_items(otn_raw_p, rb_p, tglob_p):
                    # item 0 applies the deferred 1/sums scaling (rb arrives
                    # with a full group of slack); items 1..8 do the
                    # out-projection matmuls + drains + stores
                    items = []

                    def scale_item(otn_raw_p=otn_raw_p, rb_p=rb_p,
                                   tglob_p=tglob_p):
                        otn_s = ot_sb_pool.tile([128, T_BLOCK], BF16,
                                                tag="otn_s",
                                                name=f"otns{tglob_p}")
                        nc.vector.tensor_mul(out=otn_s[:], in0=otn_raw_p[:],
                                             in1=rb_p[:])
                        os_cur['otn'] = otn_s
                    items.append(scale_item)
                    for k in range(NJ):
                        for h2 in range(2):
                            def item(k=k, h2=h2, tglob_p=tglob_p):
                                otn_s = os_cur['otn']
                                if h2 == 0:
                                    os_cur['t'] = out_pool.tile(
                                        [128, E], BF16, tag="osb",
                                        name=f"osb{tglob_p}_{k}")
                                os_t = os_cur['t']
                                wp = wo_ps.tile([128, 512], F32, tag="wo",
                                                name=f"wop{tglob_p}_{k}_{h2}")
                                nc.tensor.matmul(
                                    wp[:], otn_s[:, k * 128:(k + 1) * 128],
                                    wo_sb[:, h2 * 512:(h2 + 1) * 512],
                                    start=True, stop=True)
                                if h2 == 0:
                                    nc.vector.tensor_copy(
                                        out=os_t[:, 0:512], in_=wp[:])
                                else:
                                    nc.scalar.copy(
                                        out=os_t[:, 512:1024], in_=wp[:])
                                if h2 == 1:
                                    nc.sync.dma_start(
                                        out=out_partial[tglob_p + k * 128:
                                                        tglob_p + (k + 1) * 128, :],
                                        in_=os_t[:])
                            items.append(item)
                    return items

                for gi, (b, tb) in enumerate(groups):
                    tglob = b * T + tb * T_BLOCK
                    emit_bias_dma(gi + 2)
                    bt = bias_tiles.pop(gi)

                    ots = [ot_ps.tile([128, T_BLOCK], F32, tag="ot",
                                      name=f"ot{b}_{tb}_{a}") for a in range(HPC)]

                    def emit_pv(pend):
                        for a, e_sl, pst in pend:
                            nc.tensor.matmul(
                                ots[a][:],
                                V_sbb[b][:, pst, a * 128:a * 128 + 128],
                                e_sl,
                                start=(pst == 0), stop=(pst == NST - 1))

                    pendq = []
                    for sp in range(NST // 2):
                        sts = [st_ps.tile([128, 2, T_BLOCK], F32, tag="st",
                                          name=f"st{b}_{tb}_{sp}_{a}")
                               for a in range(HPC)]
                        for a in range(HPC):
                            r0 = a * D
                            pe_unit = ASSIGN[sp * 2 + a] == 'p'
                            for half in range(2):
                                st_i = sp * 2 + half
                                nc.tensor.matmul(
                                    sts[a][:, half, :],
                                    KTb[b][r0:r0 + D, st_i * 128:(st_i + 1) * 128],
                                    QTb[b][r0:r0 + D,
                                           tb * T_BLOCK:(tb + 1) * T_BLOCK],
                                    start=True, stop=not pe_unit)
                        if len(pendq) >= 2:
                            emit_pv(pendq.pop(0))
                        if pending_wo and sp >= 2:
                            pending_wo.pop(0)()
                        pend = []
                        for a in range(HPC):
                            pe_unit = ASSIGN[sp * 2 + a] == 'p'
                            e_t = e_pool.tile([128, 2, T_BLOCK], BF16, tag="et",
                                              name=f"et{b}_{tb}_{sp}_{a}")
                            if pe_unit:
                                for half in range(2):
                                    st_i = sp * 2 + half
                                    nc.tensor.matmul(
                                        sts[a][:, half, :], i_bf[:],
                                        bt[a][:, st_i, :],
                                        start=False, stop=True)
                                nc.scalar.activation(
                                    out=e_t[:], in_=sts[a][:],
                                    func=mybir.ActivationFunctionType.Exp)
                            else:
                                stg = stage_pool.tile([128, 2, T_BLOCK], F32,
                                                      tag="stg",
                                                      name=f"stg{b}_{tb}_{sp}_{a}")
                                nc.vector.tensor_add(
                                    out=stg[:], in0=sts[a][:],
                                    in1=bt[a][:, sp * 2:sp * 2 + 2, :])
                                nc.scalar.activation(
                                    out=e_t[:], in_=stg[:],
                                    func=mybir.ActivationFunctionType.Exp)
                            for half in range(2):
                                pend.append((a, e_t[:, half, :], sp * 2 + half))
                        pendq.append(pend)
                    for pend in pendq:
                        emit_pv(pend)
                    while pending_wo:
                        pending_wo.pop(0)()

                    # per head: drain O.T with a PLAIN copy (frees ot PSUM
                    # without waiting on 1/sums); r = exp(-ln(sums)) on
                    # ScalarE; rb broadcast via DRAM bounce consumed by the
                    # deferred scale item during the NEXT group
                    otn_raw = ot_sb_pool.tile([128, T_BLOCK], BF16,
                                              tag="otn_raw", name=f"otnr{b}_{tb}")
                    rb = rb_pool.tile([128, T_BLOCK], F32, tag="rb",
                                      name=f"rb{b}_{tb}")
                    for a in range(HPC):
                        ls = r_pool.tile([1, T_BLOCK], F32, tag="ls",
                                         name=f"ls{b}_{tb}_{a}")
                        nc.scalar.activation(out=ls[:], in_=ots[a][D:D + 1, :],
                                             func=mybir.ActivationFunctionType.Ln)
                        rr = r_pool.tile([1, T_BLOCK], F32, tag="rr",
                                         name=f"rr{b}_{tb}_{a}")
                        nc.scalar.activation(out=rr[:], in_=ls[:],
                                             func=mybir.ActivationFunctionType.Exp,
                                             scale=-1.0)
                        nc.vector.tensor_copy(out=otn_raw[a * D:(a + 1) * D, :],
                                              in_=ots[a][0:D, :])
                        idx = gi * HPC + a
                        nc.sync.dma_start(out=rc_dram[idx, :], in_=rr[:])
                        src = bass.AP(rc_dram[:].tensor, idx * T_BLOCK,
                                      [[0, D], [1, T_BLOCK]])
                        nc.sync.dma_start(out=rb[a * D:(a + 1) * D, :], in_=src)
                    pending_wo = make_wo_items(otn_raw, rb, tglob)
                while pending_wo:
                    pending_wo.pop(0)()
    _waitfix(nc)
    return nc


# ---------------- host-side prep ----------------

def shard_inputs(hidden_states, attn_bias, attention_mask, Wq, bq, Wk, bk, Wv, bv,
                 Wo, bo, c_attn, n_cores=8, scaling=None):
    """Build per-core input maps. Returns (in_maps, with_mask=False); the
    attention mask (when nonzero) is folded into the bias on the host."""
    import ml_dtypes
    bf16 = ml_dtypes.bfloat16
    B, T, E = hidden_states.shape
    NH = c_attn.shape[0]
    D = E // NH
    HPC = NH // n_cores
    DH = HPC * D

    hsT = np.ascontiguousarray(hidden_states.reshape(B * T, E).T).astype(bf16)
    bias4 = attn_bias.reshape(B, NH, T, T)
    if np.any(attention_mask):
        bias4 = bias4 + attention_mask.reshape(B, 1, T, T)

    if scaling is None:
        scaling = float(D * 2.0) ** -0.5

    in_maps = []
    for c in range(n_cores):
        r0 = c * DH
        sl = slice(r0, r0 + DH)
        cvec = np.repeat(c_attn[c * HPC:(c + 1) * HPC], D)
        # [B, HPC, NTB, p, st, u]: biasT6[b,a,tb,p,st,u] = bias[b, head,
        # t=tb*512+u, s=st*128+p] — matches the SBUF tile layout exactly so
        # each bias DMA reads 16KB contiguous per partition
        bh = bias4[:, c * HPC:(c + 1) * HPC]
        biasTc = np.ascontiguousarray(
            bh.reshape(B, HPC, T // 512, 512, T // 128, 128)
            .transpose(0, 1, 2, 5, 4, 3)).astype(bf16)
        m = {
            "hsT": hsT,
            "wqT": np.ascontiguousarray((Wq[sl] * scaling).T).astype(bf16),
            "wkT": np.ascontiguousarray(Wk[sl].T).astype(bf16),
            "wvT": np.ascontiguousarray((Wv[sl] * cvec[:, None]).T).astype(bf16),
            "bq": np.ascontiguousarray((bq[sl] * scaling)[:, None]).astype(np.float32),
            "bk": np.ascontiguousarray(bk[sl][:, None]).astype(np.float32),
            "bv": np.ascontiguousarray((bv[sl] * cvec)[:, None]).astype(np.float32),
            "woT": np.ascontiguousarray(Wo[:, sl].T).astype(bf16),
            "biasT": biasTc,
        }
        in_maps.append(m)
    return in_maps, False


_NC_CACHE = {}


def run_spmd(in_maps, with_mask=False, **kwargs):
    if "v2" not in _NC_CACHE:
        _NC_CACHE["v2"] = build_attention_nc(B=B, T=T, E=E, HPC=HPC, D=D)
    nc = _NC_CACHE["v2"]
    return run_bass_kernel_spmd(nc, in_maps, list(range(N_CORES)), **kwargs)


def kernel(hidden_states, attn_bias, attention_mask, Wq, bq, Wk, bk, Wv, bv,
           Wo, bo, c_attn):
    args = [np.asarray(a, dtype=np.float32) for a in
            (hidden_states, attn_bias, attention_mask, Wq, bq, Wk, bk, Wv, bv,
             Wo, bo, c_attn)]
    (hidden_states, attn_bias, attention_mask, Wq, bq, Wk, bk, Wv, bv,
     Wo, bo, c_attn) = args
    in_maps, with_mask = shard_inputs(hidden_states, attn_bias, attention_mask,
                                      Wq, bq, Wk, bk, Wv, bv, Wo, bo, c_attn,
                                      n_cores=N_CORES, scaling=SCALING)
    res = run_spmd(in_maps, with_mask)
    out = np.zeros((B * T, E), np.float32)
    for r in res.results:
        out += r["out"]
    out += bo[None, :]
    return out.reshape(B, T, E).astype(np.float32)


# revision 23
# speedup vs baseline: 1.2447x; 1.0553x over previous
"""OFA attention (dense_transformer) on 8 Trainium2 NeuronCores.

Sharding: heads split over cores (core c owns heads {2c, 2c+1}, both batches).
Per-core Bass/Tile program (build_attention_nc):
  phase 1 : QT/KT/VT = W_c @ hs.T (transposed projections; SCALING folded into Wq,
            c_attn folded into Wv on host; bias-add fused into PSUM drain on ScalarE)
  phase 1b: V natural = PE-transpose(VT), packed [V_A | 1 | V_B | 1] bf16
  phase 2 : per (batch, 512-token t-block), streaming 128-row s-tiles:
              ST(s,t) = K Q^T            (row-tiled K=64 matmuls)
              ST += bias.T               (bias pre-transposed to [s,t] bf16 on HOST;
                                          adds split: some s-tiles accumulate via a
                                          resident-identity matmul on PE, the rest on
                                          DVE tensor_add into an f32 staging tile)
              E = exp(ST)                (ScalarE only; exp without max-subtraction:
                                          scores+bias stay in [-8, 8])
              [O.T ; sums] += [V|1].T@E  (PV matmul also produces softmax denominators)
            per head: r = 1/sums (DVE), partition_broadcast (GpSimd), O.T drain fused
            with the r scaling (DVE tensor_mul) -> out-projection runs at full
            contract-128 (both heads in one matmul), spread across the next group's
            sp iterations; drains split DVE/GpSimd.
Host: partial outputs summed over cores + bo (the "all-reduce" of the out-projection).
"""
import sys

for _p in ("/opt/trn_rl_repo",):
    if _p not in sys.path:
        sys.path.append(_p)

import numpy as np

import concourse.bass as bass
import concourse.tile as tile
from concourse import mybir
from concourse.masks import make_identity
from concourse.bass_utils import run_bass_kernel_spmd

F32 = mybir.dt.float32
BF16 = mybir.dt.bfloat16

B, T, E, NH, D = 2, 2048, 1024, 16, 64
N_CORES = 8
HPC = NH // N_CORES
DH = HPC * D
SCALING = float(D * 2.0) ** -0.5

# Bias-add engine per unit u = sp*2 + a: 'p' = PE identity-matmul, 'v' = DVE.
# (GpSimd cannot read PSUM, so it only does broadcasts + bias DMA triggers.)
ASSIGN = ['p', 'v', 'v', 'v', 'p', 'v', 'v', 'v',
          'v', 'p', 'v', 'v', 'v', 'p', 'v', 'v']


def _waitfix(nc, limit=1):
    """This walrus build accepts at most ONE sync-wait per instruction.
    Hoist excess sem-waits onto inserted single-wait NoOps."""
    n_fixed = 0
    for bb in nc.m.functions[0].blocks:
        i = 0
        insts = bb.instructions
        while i < len(insts):
            inst = insts[i]
            si = inst.sync_info
            if si and si.on_wait and len(si.on_wait) > limit:
                extra = si.on_wait[limit:]
                si.on_wait = si.on_wait[:limit]
                for k, w in enumerate(extra):
                    nop = mybir.InstNoOp(
                        name=f"{inst.name}-waitfix{k}",
                        engine=inst.engine,
                        sync_info=mybir.SyncInfo(on_wait=[w], on_update=[]),
                        bass_nofuse=True,
                    )
                    nc.register_instruction(nop, overwrite=True)
                    insts.insert(i, nop)
                    i += 1
                n_fixed += 1
            i += 1
    return n_fixed


def build_attention_nc(B=2, T=2048, E=1024, HPC=2, D=64,
                       T_BLOCK=512, PROJ_BLOCK=512):
    """Build the per-core Bass program. Returns nc."""
    S = T
    PROJ_BLOCK = min(PROJ_BLOCK, T)
    TOK = B * T
    DH = HPC * D                      # 128
    assert DH == 128 and D == 64 and HPC == 2
    NE = E // 128                     # e-tiles
    NST = S // 128                    # s-tiles per batch
    NTB = T // T_BLOCK                # t-blocks per batch
    NJ = T_BLOCK // 128               # t-subtiles per block
    NPB = TOK // PROJ_BLOCK           # proj token blocks

    nc = bass.Bass()

    hsT = nc.declare_dram_parameter("hsT", [E, TOK], BF16, isOutput=False)
    wqT = nc.declare_dram_parameter("wqT", [E, DH], BF16, isOutput=False)
    wkT = nc.declare_dram_parameter("wkT", [E, DH], BF16, isOutput=False)
    wvT = nc.declare_dram_parameter("wvT", [E, DH], BF16, isOutput=False)
    bq = nc.declare_dram_parameter("bq", [DH, 1], F32, isOutput=False)
    bk = nc.declare_dram_parameter("bk", [DH, 1], F32, isOutput=False)
    bv = nc.declare_dram_parameter("bv", [DH, 1], F32, isOutput=False)
    woT = nc.declare_dram_parameter("woT", [DH, E], BF16, isOutput=False)
    biasT_in = nc.declare_dram_parameter(
        "biasT", [B, HPC, T // T_BLOCK, 128, S // 128, T_BLOCK], BF16,
        isOutput=False)
    out_partial = nc.declare_dram_parameter("out", [TOK, E], BF16, isOutput=True)
    rc_dram = nc.dram_tensor("rc_scratch", [B * NTB * HPC, T_BLOCK], F32)

    with tile.TileContext(nc) as tc:
        from contextlib import ExitStack
        with ExitStack() as ctx:
            consts = ctx.enter_context(tc.tile_pool(name="consts", bufs=1))
            persist = ctx.enter_context(tc.tile_pool(name="persist", bufs=1))
            bias_pool = ctx.enter_context(tc.tile_pool(name="bias_sb", bufs=2))

            i_bf = consts.tile([128, 128], BF16, tag="i_bf")
            make_identity(nc, i_bf[:])

            # weights: (E, DH) -> (128, NE, DH), bf16
            w_sb = {}
            for name, src in (("wq", wqT), ("wk", wkT), ("wv", wvT)):
                t = consts.tile([128, NE, DH], BF16, tag=name)
                nc.gpsimd.dma_start(out=t[:], in_=src.rearrange("(n p) d -> p n d", p=128))
                w_sb[name] = t
            wo_sb = consts.tile([128, E], BF16, tag="wo")
            nc.gpsimd.dma_start(out=wo_sb[:], in_=woT[:, :])
            b_sb = {}
            for name, src in (("bq", bq), ("bk", bk), ("bv", bv)):
                t = consts.tile([128, 1], F32, tag=name)
                nc.gpsimd.dma_start(out=t[:], in_=src[:, :])
                b_sb[name] = t

            # persistent activations
            QTb = [persist.tile([128, T], BF16, tag=f"QT{bb}", name=f"QT{bb}")
                   for bb in range(B)]
            KTb = [persist.tile([128, T], BF16, tag=f"KT{bb}", name=f"KT{bb}")
                   for bb in range(B)]
            VTb = [persist.tile([128, T], BF16, tag=f"VT{bb}", name=f"VT{bb}")
                   for bb in range(B)]
            V_sbb = []
            for bb in range(B):
                V_sb = persist.tile([128, T // 128, 256], BF16, tag=f"V_sb{bb}",
                                    name=f"V_sb{bb}")
                nc.vector.memset(V_sb[:, :, :], 0.0)
                nc.vector.memset(V_sb[:, :, D:D + 1], 1.0)
                nc.vector.memset(V_sb[:, :, 128 + D:128 + D + 1], 1.0)
                V_sbb.append(V_sb)

            # group list + bias prefetch plumbing (1-group emission lookahead;
            # DMAs ride the otherwise-idle GpSimd queue, transfers overlap the
            # previous group / phase 1)
            groups = [(b, tb) for b in range(B) for tb in range(NTB)]
            bias_tiles = {}

            def emit_bias_dma(gi):
                if gi >= len(groups):
                    return
                b, tb = groups[gi]
                bts = []
                for a in range(HPC):
                    t_ = bias_pool.tile([128, NST, T_BLOCK], BF16,
                                        tag=f"bias{a}", name=f"bias{b}_{tb}_{a}")
                    nc.gpsimd.dma_start(out=t_[:], in_=biasT_in[b, a, tb])
                    bts.append(t_)
                bias_tiles[gi] = bts

            # ---------------- phase 1: projections ----------------
            # all input loads ride the gpsimd queue IN ORDER (weights, then
            # hsT strips, then bias prefetch) so bias cannot starve phase 1
            with tc.tile_pool(name="hst", bufs=1) as hst_pool, \
                 tc.tile_pool(name="proj_ps", bufs=3, space="PSUM") as proj_ps:
                hstrips = {}
                for bb2 in range(B):
                    for q in range(T // 512):
                        for e in range(NE):
                            if q == 0:
                                h = hst_pool.tile([128, T], BF16,
                                                  tag=f"hst{bb2}_{e}",
                                                  name=f"hst{bb2}_{e}")
                                hstrips[(bb2, e)] = h
                            h = hstrips[(bb2, e)]
                            nc.gpsimd.dma_start(
                                out=h[:, q * 512:(q + 1) * 512],
                                in_=hsT[e * 128:(e + 1) * 128,
                                        bb2 * T + q * 512:bb2 * T + (q + 1) * 512])
                emit_bias_dma(0)
                emit_bias_dma(1)
                for pb in range(NPB):
                    t0 = pb * PROJ_BLOCK
                    bb = t0 // T
                    tloc = t0 % T
                    for name, dstl in (("wq", QTb), ("wk", KTb), ("wv", VTb)):
                        ps = proj_ps.tile([128, PROJ_BLOCK], F32, tag="proj",
                                          name=f"pps{pb}_{name}")
                        for e in range(NE):
                            nc.tensor.matmul(ps[:], w_sb[name][:, e, :],
                                             hstrips[(bb, e)][:, tloc:tloc + PROJ_BLOCK],
                                             start=(e == 0), stop=(e == NE - 1))
                        nc.vector.tensor_scalar_add(
                            out=dstl[bb][:, tloc:tloc + PROJ_BLOCK], in0=ps[:],
                            scalar1=b_sb["b" + name[1]][:])

            # ---------------- phase 1b: V natural ----------------
            with tc.tile_pool(name="vtr_ps", bufs=2, space="PSUM") as vtr_ps:
                for bb in range(B):
                    for st in range(T // 128):
                        ps = vtr_ps.tile([128, 128], BF16, tag="vtr",
                                         name=f"vtr{bb}_{st}")
                        nc.tensor.transpose(ps[:], VTb[bb][:, st * 128:(st + 1) * 128],
                                            i_bf[:])
                        nc.vector.tensor_copy(out=V_sbb[bb][:, st, 0:D],
                                              in_=ps[:, 0:D])
                        nc.vector.tensor_copy(out=V_sbb[bb][:, st, 128:128 + D],
                                              in_=ps[:, D:2 * D])

            # ---------------- phase 2: attention ----------------
            with tc.tile_pool(name="stage", bufs=4) as stage_pool, \
                 tc.tile_pool(name="e_sb", bufs=6) as e_pool, \
                 tc.tile_pool(name="ot_sb", bufs=2) as ot_sb_pool, \
                 tc.tile_pool(name="rr", bufs=4) as r_pool, \
                 tc.tile_pool(name="rb", bufs=4) as rb_pool, \
                 tc.tile_pool(name="osb", bufs=3) as out_pool, \
                 tc.tile_pool(name="st_ps", bufs=2, space="PSUM") as st_ps, \
                 tc.tile_pool(name="ot_ps", bufs=2, space="PSUM") as ot_ps, \
                 tc.tile_pool(name="wo_ps", bufs=2, space="PSUM") as wo_ps:

                # out-projection of the PREVIOUS group, emitted one (k, half)
                # item per sp iteration so the wo PSUM ring never stalls PE
                pending_wo = []   # list of closures
                os_cur = {}

                def make_wo_items(otn_raw_p, rb_p, tglob_p):
                    # item 0 applies the deferred 1/sums scaling (rb arrives
                    # with a full group of slack); items 1..8 do the
                    # out-projection matmuls + drains + stores
                    items = []

                    def scale_item(otn_raw_p=otn_raw_p, rb_p=rb_p,
                                   tglob_p=tglob_p):
                        otn_s = ot_sb_pool.tile([128, T_BLOCK], BF16,
                                                tag="otn_s",
                                                name=f"otns{tglob_p}")
                        nc.vector.tensor_mul(out=otn_s[:], in0=otn_raw_p[:],
                                             in1=rb_p[:])
                        os_cur['otn'] = otn_s
                    items.append(scale_item)
                    for k in range(NJ):
                        for h2 in range(2):
                            def item(k=k, h2=h2, tglob_p=tglob_p):
                                otn_s = os_cur['otn']
                                if h2 == 0:
                                    os_cur['t'] = out_pool.tile(
                                        [128, E], BF16, tag="osb",
                                        name=f"osb{tglob_p}_{k}")
                                os_t = os_cur['t']
                                wp = wo_ps.tile([128, 512], F32, tag="wo",
                                                name=f"wop{tglob_p}_{k}_{h2}")
                                nc.tensor.matmul(
                                    wp[:], otn_s[:, k * 128:(k + 1) * 128],
                                    wo_sb[:, h2 * 512:(h2 + 1) * 512],
                                    start=True, stop=True)
                                if h2 == 0:
                                    nc.vector.tensor_copy(
                                        out=os_t[:, 0:512], in_=wp[:])
                                else:
                                    nc.scalar.copy(
                                        out=os_t[:, 512:1024], in_=wp[:])
                                if h2 == 1:
                                    nc.sync.dma_start(
                                        out=out_partial[tglob_p + k * 128:
                                                        tglob_p + (k + 1) * 128, :],
                                        in_=os_t[:])
                            items.append(item)
                    return items

                for gi, (b, tb) in enumerate(groups):
                    tglob = b * T + tb * T_BLOCK
                    emit_bias_dma(gi + 2)
                    bt = bias_tiles.pop(gi)

                    ots = [ot_ps.tile([128, T_BLOCK], F32, tag="ot",
                                      name=f"ot{b}_{tb}_{a}") for a in range(HPC)]

                    def emit_pv(pend):
                        for a, e_sl, pst in pend:
                            nc.tensor.matmul(
                                ots[a][:],
                                V_sbb[b][:, pst, a * 128:a * 128 + 128],
                                e_sl,
                                start=(pst == 0), stop=(pst == NST - 1))

                    pendq = []
                    for sp in range(NST // 2):
                        sts = [st_ps.tile([128, 2, T_BLOCK], F32, tag="st",
                                          name=f"st{b}_{tb}_{sp}_{a}")
                               for a in range(HPC)]
                        for a in range(HPC):
                            r0 = a * D
                            pe_unit = ASSIGN[sp * 2 + a] == 'p'
                            for half in range(2):
                                st_i = sp * 2 + half
                                nc.tensor.matmul(
                                    sts[a][:, half, :],
                                    KTb[b][r0:r0 + D, st_i * 128:(st_i + 1) * 128],
                                    QTb[b][r0:r0 + D,
                                           tb * T_BLOCK:(tb + 1) * T_BLOCK],
                                    start=True, stop=not pe_unit)
                        if len(pendq) >= 2:
                            emit_pv(pendq.pop(0))
                        if pending_wo and sp >= 2:
                            pending_wo.pop(0)()
                        pend = []
                        for a in range(HPC):
                            pe_unit = ASSIGN[sp * 2 + a] == 'p'
                            e_t = e_pool.tile([128, 2, T_BLOCK], BF16, tag="et",
                                              name=f"et{b}_{tb}_{sp}_{a}")
                            if pe_unit:
                                for half in range(2):
                                    st_i = sp * 2 + half
                                    nc.tensor.matmul(
                                        sts[a][:, half, :], i_bf[:],
                                        bt[a][:, st_i, :],
                                        start=False, stop=True)
                                nc.scalar.activation(
                                    out=e_t[:], in_=sts[a][:],
                                    func=mybir.ActivationFunctionType.Exp)
                            else:
                                stg = stage_pool.tile([128, 2, T_BLOCK], F32,
                                                      tag="stg",
                                                      name=f"stg{b}_{tb}_{sp}_{a}")
                                nc.vector.tensor_add(
                                    out=stg[:], in0=sts[a][:],
                                    in1=bt[a][:, sp * 2:sp * 2 + 2, :])
                                nc.scalar.activation(
                                    out=e_t[:], in_=stg[:],
                                    func=mybir.ActivationFunctionType.Exp)
                            for half in range(2):
                                pend.append((a, e_t[:, half, :], sp * 2 + half))
                        pendq.append(pend)
                    for pend in pendq:
                        emit_pv(pend)
                    while pending_wo:
                        pending_wo.pop(0)()

                    # per head: drain O.T with a PLAIN copy (frees ot PSUM
                    # without waiting on 1/sums); r = exp(-ln(sums)) on
                    # ScalarE; rb broadcast via DRAM bounce consumed by the
                    # deferred scale item during the NEXT group
                    otn_raw = ot_sb_pool.tile([128, T_BLOCK], BF16,
                                              tag="otn_raw", name=f"otnr{b}_{tb}")
                    rb = rb_pool.tile([128, T_BLOCK], F32, tag="rb",
                                      name=f"rb{b}_{tb}")
                    for a in range(HPC):
                        ls = r_pool.tile([1, T_BLOCK], F32, tag="ls",
                                         name=f"ls{b}_{tb}_{a}")
                        nc.scalar.activation(out=ls[:], in_=ots[a][D:D + 1, :],
                                             func=mybir.ActivationFunctionType.Ln)
                        rr = r_pool.tile([1, T_BLOCK], F32, tag="rr",
                                         name=f"rr{b}_{tb}_{a}")
                        nc.scalar.activation(out=rr[:], in_=ls[:],
                                             func=mybir.ActivationFunctionType.Exp,
                                             scale=-1.0)
                        nc.vector.tensor_copy(out=otn_raw[a * D:(a + 1) * D, :],
                                              in_=ots[a][0:D, :])
                        idx = gi * HPC + a
                        nc.sync.dma_start(out=rc_dram[idx, :], in_=rr[:])
                        src = bass.AP(rc_dram[:].tensor, idx * T_BLOCK,
                                      [[0, D], [1, T_BLOCK]])
                        nc.sync.dma_start(out=rb[a * D:(a + 1) * D, :], in_=src)
                    pending_wo = make_wo_items(otn_raw, rb, tglob)
                while pending_wo:
                    pending_wo.pop(0)()
    _waitfix(nc)
    return nc


# ---------------- host-side prep ----------------

def shard_inputs(hidden_states, attn_bias, attention_mask, Wq, bq, Wk, bk, Wv, bv,
                 Wo, bo, c_attn, n_cores=8, scaling=None):
    """Build per-core input maps. Returns (in_maps, with_mask=False); the
    attention mask (when nonzero) is folded into the bias on the host."""
    import ml_dtypes
    bf16 = ml_dtypes.bfloat16
    B, T, E = hidden_states.shape
    NH = c_attn.shape[0]
    D = E // NH
    HPC = NH // n_cores
    DH = HPC * D

    hsT = np.ascontiguousarray(hidden_states.reshape(B * T, E).T).astype(bf16)
    bias4 = attn_bias.reshape(B, NH, T, T)
    if np.any(attention_mask):
        bias4 = bias4 + attention_mask.reshape(B, 1, T, T)

    if scaling is None:
        scaling = float(D * 2.0) ** -0.5

    in_maps = []
    for c in range(n_cores):
        r0 = c * DH
        sl = slice(r0, r0 + DH)
        cvec = np.repeat(c_attn[c * HPC:(c + 1) * HPC], D)
        # [B, HPC, NTB, p, st, u]: biasT6[b,a,tb,p,st,u] = bias[b, head,
        # t=tb*512+u, s=st*128+p] — matches the SBUF tile layout exactly so
        # each bias DMA reads 16KB contiguous per partition
        bh = bias4[:, c * HPC:(c + 1) * HPC]
        biasTc = np.ascontiguousarray(
            bh.reshape(B, HPC, T // 512, 512, T // 128, 128)
            .transpose(0, 1, 2, 5, 4, 3)).astype(bf16)
        m = {
            "hsT": hsT,
            "wqT": np.ascontiguousarray((Wq[sl] * scaling).T).astype(bf16),
            "wkT": np.ascontiguousarray(Wk[sl].T).astype(bf16),
            "wvT": np.ascontiguousarray((Wv[sl] * cvec[:, None]).T).astype(bf16),
            "bq": np.ascontiguousarray((bq[sl] * scaling)[:, None]).astype(np.float32),
            "bk": np.ascontiguousarray(bk[sl][:, None]).astype(np.float32),
            "bv": np.ascontiguousarray((bv[sl] * cvec)[:, None]).astype(np.float32),
            "woT": np.ascontiguousarray(Wo[:, sl].T).astype(bf16),
            "biasT": biasTc,
        }
        in_maps.append(m)
    return in_maps, False


_NC_CACHE = {}


def run_spmd(in_maps, with_mask=False, **kwargs):
    if "v2" not in _NC_CACHE:
        _NC_CACHE["v2"] = build_attention_nc(B=B, T=T, E=E, HPC=HPC, D=D)
    nc = _NC_CACHE["v2"]
    return run_bass_kernel_spmd(nc, in_maps, list(range(N_CORES)), **kwargs)


def kernel(hidden_states, attn_bias, attention_mask, Wq, bq, Wk, bk, Wv, bv,
           Wo, bo, c_attn):
    args = [np.asarray(a, dtype=np.float32) for a in
            (hidden_states, attn_bias, attention_mask, Wq, bq, Wk, bk, Wv, bv,
             Wo, bo, c_attn)]
    (hidden_states, attn_bias, attention_mask, Wq, bq, Wk, bk, Wv, bv,
     Wo, bo, c_attn) = args
    in_maps, with_mask = shard_inputs(hidden_states, attn_bias, attention_mask,
                                      Wq, bq, Wk, bk, Wv, bv, Wo, bo, c_attn,
                                      n_cores=N_CORES, scaling=SCALING)
    res = run_spmd(in_maps, with_mask)
    out = np.zeros((B * T, E), np.float32)
    for r in res.results:
        out += r["out"]
    out += bo[None, :]
    return out.reshape(B, T, E).astype(np.float32)


# revision 24
# speedup vs baseline: 1.2609x; 1.0131x over previous
"""OFA attention (dense_transformer) on 8 Trainium2 NeuronCores.

Sharding: heads split over cores (core c owns heads {2c, 2c+1}, both batches).
Per-core Bass/Tile program (build_attention_nc):
  phase 1 : QT/KT/VT = W_c @ hs.T (transposed projections; SCALING folded into Wq,
            c_attn folded into Wv on host; bias-add fused into PSUM drain on ScalarE)
  phase 1b: V natural = PE-transpose(VT), packed [V_A | 1 | V_B | 1] bf16
  phase 2 : per (batch, 512-token t-block), streaming 128-row s-tiles:
              ST(s,t) = K Q^T            (row-tiled K=64 matmuls)
              ST += bias.T               (bias pre-transposed to [s,t] bf16 on HOST;
                                          adds split: some s-tiles accumulate via a
                                          resident-identity matmul on PE, the rest on
                                          DVE tensor_add into an f32 staging tile)
              E = exp(ST)                (ScalarE only; exp without max-subtraction:
                                          scores+bias stay in [-8, 8])
              [O.T ; sums] += [V|1].T@E  (PV matmul also produces softmax denominators)
            per head: r = 1/sums (DVE), partition_broadcast (GpSimd), O.T drain fused
            with the r scaling (DVE tensor_mul) -> out-projection runs at full
            contract-128 (both heads in one matmul), spread across the next group's
            sp iterations; drains split DVE/GpSimd.
Host: partial outputs summed over cores + bo (the "all-reduce" of the out-projection).
"""
import sys

for _p in ("/opt/trn_rl_repo",):
    if _p not in sys.path:
        sys.path.append(_p)

import numpy as np

import concourse.bass as bass
import concourse.tile as tile
from concourse import mybir
from concourse.masks import make_identity
from concourse.bass_utils import run_bass_kernel_spmd

F32 = mybir.dt.float32
BF16 = mybir.dt.bfloat16

B, T, E, NH, D = 2, 2048, 1024, 16, 64
N_CORES = 8
HPC = NH // N_CORES
DH = HPC * D
SCALING = float(D * 2.0) ** -0.5

# Bias-add engine per unit u = sp*2 + a: 'p' = PE identity-matmul, 'v' = DVE.
# (GpSimd cannot read PSUM, so it only does broadcasts + bias DMA triggers.)
ASSIGN = ['p', 'v', 'v', 'v', 'p', 'v', 'v', 'v',
          'v', 'p', 'v', 'v', 'v', 'p', 'v', 'v']


def _waitfix(nc, limit=1):
    """This walrus build accepts at most ONE sync-wait per instruction.
    Hoist excess sem-waits onto inserted single-wait NoOps."""
    n_fixed = 0
    for bb in nc.m.functions[0].blocks:
        i = 0
        insts = bb.instructions
        while i < len(insts):
            inst = insts[i]
            si = inst.sync_info
            if si and si.on_wait and len(si.on_wait) > limit:
                extra = si.on_wait[limit:]
                si.on_wait = si.on_wait[:limit]
                for k, w in enumerate(extra):
                    nop = mybir.InstNoOp(
                        name=f"{inst.name}-waitfix{k}",
                        engine=inst.engine,
                        sync_info=mybir.SyncInfo(on_wait=[w], on_update=[]),
                        bass_nofuse=True,
                    )
                    nc.register_instruction(nop, overwrite=True)
                    insts.insert(i, nop)
                    i += 1
                n_fixed += 1
            i += 1
    return n_fixed


def build_attention_nc(B=2, T=2048, E=1024, HPC=2, D=64,
                       T_BLOCK=512, PROJ_BLOCK=512):
    """Build the per-core Bass program. Returns nc."""
    S = T
    PROJ_BLOCK = min(PROJ_BLOCK, T)
    TOK = B * T
    DH = HPC * D                      # 128
    assert DH == 128 and D == 64 and HPC == 2
    NE = E // 128                     # e-tiles
    NST = S // 128                    # s-tiles per batch
    NTB = T // T_BLOCK                # t-blocks per batch
    NJ = T_BLOCK // 128               # t-subtiles per block
    NPB = TOK // PROJ_BLOCK           # proj token blocks

    nc = bass.Bass()

    hsT = nc.declare_dram_parameter("hsT", [E, TOK], BF16, isOutput=False)
    wqT = nc.declare_dram_parameter("wqT", [E, DH], BF16, isOutput=False)
    wkT = nc.declare_dram_parameter("wkT", [E, DH], BF16, isOutput=False)
    wvT = nc.declare_dram_parameter("wvT", [E, DH], BF16, isOutput=False)
    bq = nc.declare_dram_parameter("bq", [DH, 1], F32, isOutput=False)
    bk = nc.declare_dram_parameter("bk", [DH, 1], F32, isOutput=False)
    bv = nc.declare_dram_parameter("bv", [DH, 1], F32, isOutput=False)
    woT = nc.declare_dram_parameter("woT", [DH, E], BF16, isOutput=False)
    biasT_in = nc.declare_dram_parameter(
        "biasT", [B, HPC, T // T_BLOCK, 128, S // 128, T_BLOCK], BF16,
        isOutput=False)
    out_partial = nc.declare_dram_parameter("out", [TOK, E], BF16, isOutput=True)
    rc_dram = nc.dram_tensor("rc_scratch", [B * NTB * HPC, T_BLOCK], F32)

    with tile.TileContext(nc) as tc:
        from contextlib import ExitStack
        with ExitStack() as ctx:
            consts = ctx.enter_context(tc.tile_pool(name="consts", bufs=1))
            persist = ctx.enter_context(tc.tile_pool(name="persist", bufs=1))
            bias_pool = ctx.enter_context(tc.tile_pool(name="bias_sb", bufs=2))

            i_bf = consts.tile([128, 128], BF16, tag="i_bf")
            make_identity(nc, i_bf[:])

            # weights: (E, DH) -> (128, NE, DH), bf16
            w_sb = {}
            for name, src in (("wq", wqT), ("wk", wkT), ("wv", wvT)):
                t = consts.tile([128, NE, DH], BF16, tag=name)
                nc.gpsimd.dma_start(out=t[:], in_=src.rearrange("(n p) d -> p n d", p=128))
                w_sb[name] = t
            wo_sb = consts.tile([128, E], BF16, tag="wo")
            nc.gpsimd.dma_start(out=wo_sb[:], in_=woT[:, :])
            b_sb = {}
            for name, src in (("bq", bq), ("bk", bk), ("bv", bv)):
                t = consts.tile([128, 1], F32, tag=name)
                nc.gpsimd.dma_start(out=t[:], in_=src[:, :])
                b_sb[name] = t

            # persistent activations
            QTb = [persist.tile([128, T], BF16, tag=f"QT{bb}", name=f"QT{bb}")
                   for bb in range(B)]
            KTb = [persist.tile([128, T], BF16, tag=f"KT{bb}", name=f"KT{bb}")
                   for bb in range(B)]
            VTb = [persist.tile([128, T], BF16, tag=f"VT{bb}", name=f"VT{bb}")
                   for bb in range(B)]
            V_sbb = []
            for bb in range(B):
                V_sb = persist.tile([128, T // 128, 256], BF16, tag=f"V_sb{bb}",
                                    name=f"V_sb{bb}")
                nc.vector.memset(V_sb[:, :, :], 0.0)
                nc.vector.memset(V_sb[:, :, D:D + 1], 1.0)
                nc.vector.memset(V_sb[:, :, 128 + D:128 + D + 1], 1.0)
                V_sbb.append(V_sb)

            # group list + bias prefetch plumbing (1-group emission lookahead;
            # DMAs ride the otherwise-idle GpSimd queue, transfers overlap the
            # previous group / phase 1)
            groups = [(b, tb) for b in range(B) for tb in range(NTB)]
            bias_tiles = {}

            def emit_bias_dma(gi):
                if gi >= len(groups):
                    return
                b, tb = groups[gi]
                bts = []
                for a in range(HPC):
                    t_ = bias_pool.tile([128, NST, T_BLOCK], BF16,
                                        tag=f"bias{a}", name=f"bias{b}_{tb}_{a}")
                    nc.gpsimd.dma_start(out=t_[:], in_=biasT_in[b, a, tb])
                    bts.append(t_)
                bias_tiles[gi] = bts

            # ---------------- phase 1: projections ----------------
            # all input loads ride the gpsimd queue IN ORDER (weights, then
            # hsT strips, then bias prefetch) so bias cannot starve phase 1
            with tc.tile_pool(name="hst", bufs=1) as hst_pool, \
                 tc.tile_pool(name="proj_ps", bufs=3, space="PSUM") as proj_ps:
                hstrips = {}
                for bb2 in range(B):
                    for q in range(T // 512):
                        for e in range(NE):
                            if q == 0:
                                h = hst_pool.tile([128, T], BF16,
                                                  tag=f"hst{bb2}_{e}",
                                                  name=f"hst{bb2}_{e}")
                                hstrips[(bb2, e)] = h
                            h = hstrips[(bb2, e)]
                            nc.gpsimd.dma_start(
                                out=h[:, q * 512:(q + 1) * 512],
                                in_=hsT[e * 128:(e + 1) * 128,
                                        bb2 * T + q * 512:bb2 * T + (q + 1) * 512])
                emit_bias_dma(0)
                emit_bias_dma(1)
                for pb in range(NPB):
                    t0 = pb * PROJ_BLOCK
                    bb = t0 // T
                    tloc = t0 % T
                    for name, dstl in (("wq", QTb), ("wk", KTb), ("wv", VTb)):
                        ps = proj_ps.tile([128, PROJ_BLOCK], F32, tag="proj",
                                          name=f"pps{pb}_{name}")
                        for e in range(NE):
                            nc.tensor.matmul(ps[:], w_sb[name][:, e, :],
                                             hstrips[(bb, e)][:, tloc:tloc + PROJ_BLOCK],
                                             start=(e == 0), stop=(e == NE - 1))
                        nc.vector.tensor_scalar_add(
                            out=dstl[bb][:, tloc:tloc + PROJ_BLOCK], in0=ps[:],
                            scalar1=b_sb["b" + name[1]][:])

            # ---------------- phase 1b: V natural ----------------
            with tc.tile_pool(name="vtr_ps", bufs=2, space="PSUM") as vtr_ps:
                for bb in range(B):
                    for st in range(T // 128):
                        ps = vtr_ps.tile([128, 128], BF16, tag="vtr",
                                         name=f"vtr{bb}_{st}")
                        nc.tensor.transpose(ps[:], VTb[bb][:, st * 128:(st + 1) * 128],
                                            i_bf[:])
                        nc.vector.tensor_copy(out=V_sbb[bb][:, st, 0:D],
                                              in_=ps[:, 0:D])
                        nc.vector.tensor_copy(out=V_sbb[bb][:, st, 128:128 + D],
                                              in_=ps[:, D:2 * D])

            # ---------------- phase 2: attention ----------------
            with tc.tile_pool(name="stage", bufs=4) as stage_pool, \
                 tc.tile_pool(name="e_sb", bufs=6) as e_pool, \
                 tc.tile_pool(name="ot_sb", bufs=2) as ot_sb_pool, \
                 tc.tile_pool(name="rr", bufs=4) as r_pool, \
                 tc.tile_pool(name="rb", bufs=4) as rb_pool, \
                 tc.tile_pool(name="osb", bufs=3) as out_pool, \
                 tc.tile_pool(name="st_ps", bufs=2, space="PSUM") as st_ps, \
                 tc.tile_pool(name="ot_ps", bufs=2, space="PSUM") as ot_ps, \
                 tc.tile_pool(name="wo_ps", bufs=2, space="PSUM") as wo_ps:

                # out-projection of the PREVIOUS group, emitted one (k, half)
                # item per sp iteration so the wo PSUM ring never stalls PE
                pending_wo = []   # list of closures
                os_cur = {}

                def make_wo_items(otn_raw_p, rb_p, tglob_p):
                    # item 0 applies the deferred 1/sums scaling (rb arrives
                    # with a full group of slack); items 1..8 do the
                    # out-projection matmuls + drains + stores
                    items = []

                    def scale_item(otn_raw_p=otn_raw_p, rb_p=rb_p,
                                   tglob_p=tglob_p):
                        otn_s = ot_sb_pool.tile([128, T_BLOCK], BF16,
                                                tag="otn_s",
                                                name=f"otns{tglob_p}")
                        nc.vector.tensor_mul(out=otn_s[:], in0=otn_raw_p[:],
                                             in1=rb_p[:])
                        os_cur['otn'] = otn_s
                    items.append(scale_item)
                    for k in range(NJ):
                        for h2 in range(2):
                            def item(k=k, h2=h2, tglob_p=tglob_p):
                                otn_s = os_cur['otn']
                                if h2 == 0:
                                    os_cur['t'] = out_pool.tile(
                                        [128, E], BF16, tag="osb",
                                        name=f"osb{tglob_p}_{k}")
                                os_t = os_cur['t']
                                wp = wo_ps.tile([128, 512], F32, tag="wo",
                                                name=f"wop{tglob_p}_{k}_{h2}")
                                nc.tensor.matmul(
                                    wp[:], otn_s[:, k * 128:(k + 1) * 128],
                                    wo_sb[:, h2 * 512:(h2 + 1) * 512],
                                    start=True, stop=True)
                                if h2 == 0:
                                    nc.vector.tensor_copy(
                                        out=os_t[:, 0:512], in_=wp[:])
                                else:
                                    nc.scalar.copy(
                                        out=os_t[:, 512:1024], in_=wp[:])
                                if h2 == 1:
                                    nc.sync.dma_start(
                                        out=out_partial[tglob_p + k * 128:
                                                        tglob_p + (k + 1) * 128, :],
                                        in_=os_t[:])
                            items.append(item)
                    return items

                for gi, (b, tb) in enumerate(groups):
                    tglob = b * T + tb * T_BLOCK
                    emit_bias_dma(gi + 2)
                    bt = bias_tiles.pop(gi)

                    ots = [ot_ps.tile([128, T_BLOCK], F32, tag="ot",
                                      name=f"ot{b}_{tb}_{a}") for a in range(HPC)]

                    def emit_pv(pend):
                        for a, e_sl, pst in pend:
                            nc.tensor.matmul(
                                ots[a][:],
                                V_sbb[b][:, pst, a * 128:a * 128 + 128],
                                e_sl,
                                start=(pst == 0), stop=(pst == NST - 1))

                    pendq = []
                    for sp in range(NST // 2):
                        sts = [st_ps.tile([128, 2, T_BLOCK], F32, tag="st",
                                          name=f"st{b}_{tb}_{sp}_{a}")
                               for a in range(HPC)]
                        for a in range(HPC):
                            r0 = a * D
                            pe_unit = ASSIGN[sp * 2 + a] == 'p'
                            for half in range(2):
                                st_i = sp * 2 + half
                                nc.tensor.matmul(
                                    sts[a][:, half, :],
                                    KTb[b][r0:r0 + D, st_i * 128:(st_i + 1) * 128],
                                    QTb[b][r0:r0 + D,
                                           tb * T_BLOCK:(tb + 1) * T_BLOCK],
                                    start=True, stop=not pe_unit)
                        if len(pendq) >= 2:
                            emit_pv(pendq.pop(0))
                        if pending_wo and sp >= 2:
                            pending_wo.pop(0)()
                        pend = []
                        for a in range(HPC):
                            pe_unit = ASSIGN[sp * 2 + a] == 'p'
                            e_t = e_pool.tile([128, 2, T_BLOCK], BF16, tag="et",
                                              name=f"et{b}_{tb}_{sp}_{a}")
                            if pe_unit:
                                for half in range(2):
                                    st_i = sp * 2 + half
                                    nc.tensor.matmul(
                                        sts[a][:, half, :], i_bf[:],
                                        bt[a][:, st_i, :],
                                        start=False, stop=True)
                                nc.scalar.activation(
                                    out=e_t[:], in_=sts[a][:],
                                    func=mybir.ActivationFunctionType.Exp)
                            else:
                                stg = stage_pool.tile([128, 2, T_BLOCK], F32,
                                                      tag="stg",
                                                      name=f"stg{b}_{tb}_{sp}_{a}")
                                nc.vector.tensor_add(
                                    out=stg[:], in0=sts[a][:],
                                    in1=bt[a][:, sp * 2:sp * 2 + 2, :])
                                nc.scalar.activation(
                                    out=e_t[:], in_=stg[:],
                                    func=mybir.ActivationFunctionType.Exp)
                            for half in range(2):
                                pend.append((a, e_t[:, half, :], sp * 2 + half))
                        pendq.append(pend)
                    for pend in pendq:
                        emit_pv(pend)
                    while pending_wo:
                        pending_wo.pop(0)()

                    # per head: drain O.T with a PLAIN copy (frees ot PSUM
                    # without waiting on 1/sums); r = exp(-ln(sums)) on
                    # ScalarE; rb broadcast via DRAM bounce consumed by the
                    # deferred scale item during the NEXT group
                    otn_raw = ot_sb_pool.tile([128, T_BLOCK], BF16,
                                              tag="otn_raw", name=f"otnr{b}_{tb}")
                    rb = rb_pool.tile([128, T_BLOCK], F32, tag="rb",
                                      name=f"rb{b}_{tb}")
                    for a in range(HPC):
                        ls = r_pool.tile([1, T_BLOCK], F32, tag="ls",
                                         name=f"ls{b}_{tb}_{a}")
                        nc.scalar.activation(out=ls[:], in_=ots[a][D:D + 1, :],
                                             func=mybir.ActivationFunctionType.Ln)
                        rr = r_pool.tile([1, T_BLOCK], F32, tag="rr",
                                         name=f"rr{b}_{tb}_{a}")
                        nc.scalar.activation(out=rr[:], in_=ls[:],
                                             func=mybir.ActivationFunctionType.Exp,
                                             scale=-1.0)
                        nc.vector.tensor_copy(out=otn_raw[a * D:(a + 1) * D, :],
                                              in_=ots[a][0:D, :])
                        # bounce on the gpsimd queue: it is idle at group end
                        # (bias prefetch finishes mid-group), while the sync
                        # queue's out-store backlog would delay rb by ~7us
                        idx = gi * HPC + a
                        nc.gpsimd.dma_start(out=rc_dram[idx, :], in_=rr[:])
                        src = bass.AP(rc_dram[:].tensor, idx * T_BLOCK,
                                      [[0, D], [1, T_BLOCK]])
                        nc.gpsimd.dma_start(out=rb[a * D:(a + 1) * D, :], in_=src)
                    pending_wo = make_wo_items(otn_raw, rb, tglob)
                while pending_wo:
                    pending_wo.pop(0)()
    _waitfix(nc)
    return nc


# ---------------- host-side prep ----------------

def shard_inputs(hidden_states, attn_bias, attention_mask, Wq, bq, Wk, bk, Wv, bv,
                 Wo, bo, c_attn, n_cores=8, scaling=None):
    """Build per-core input maps. Returns (in_maps, with_mask=False); the
    attention mask (when nonzero) is folded into the bias on the host."""
    import ml_dtypes
    bf16 = ml_dtypes.bfloat16
    B, T, E = hidden_states.shape
    NH = c_attn.shape[0]
    D = E // NH
    HPC = NH // n_cores
    DH = HPC * D

    hsT = np.ascontiguousarray(hidden_states.reshape(B * T, E).T).astype(bf16)
    bias4 = attn_bias.reshape(B, NH, T, T)
    if np.any(attention_mask):
        bias4 = bias4 + attention_mask.reshape(B, 1, T, T)

    if scaling is None:
        scaling = float(D * 2.0) ** -0.5

    in_maps = []
    for c in range(n_cores):
        r0 = c * DH
        sl = slice(r0, r0 + DH)
        cvec = np.repeat(c_attn[c * HPC:(c + 1) * HPC], D)
        # [B, HPC, NTB, p, st, u]: biasT6[b,a,tb,p,st,u] = bias[b, head,
        # t=tb*512+u, s=st*128+p] — matches the SBUF tile layout exactly so
        # each bias DMA reads 16KB contiguous per partition
        bh = bias4[:, c * HPC:(c + 1) * HPC]
        biasTc = np.ascontiguousarray(
            bh.reshape(B, HPC, T // 512, 512, T // 128, 128)
            .transpose(0, 1, 2, 5, 4, 3)).astype(bf16)
        m = {
            "hsT": hsT,
            "wqT": np.ascontiguousarray((Wq[sl] * scaling).T).astype(bf16),
            "wkT": np.ascontiguousarray(Wk[sl].T).astype(bf16),
            "wvT": np.ascontiguousarray((Wv[sl] * cvec[:, None]).T).astype(bf16),
            "bq": np.ascontiguousarray((bq[sl] * scaling)[:, None]).astype(np.float32),
            "bk": np.ascontiguousarray(bk[sl][:, None]).astype(np.float32),
            "bv": np.ascontiguousarray((bv[sl] * cvec)[:, None]).astype(np.float32),
            "woT": np.ascontiguousarray(Wo[:, sl].T).astype(bf16),
            "biasT": biasTc,
        }
        in_maps.append(m)
    return in_maps, False


_NC_CACHE = {}


def run_spmd(in_maps, with_mask=False, **kwargs):
    if "v2" not in _NC_CACHE:
        _NC_CACHE["v2"] = build_attention_nc(B=B, T=T, E=E, HPC=HPC, D=D)
    nc = _NC_CACHE["v2"]
    return run_bass_kernel_spmd(nc, in_maps, list(range(N_CORES)), **kwargs)


def kernel(hidden_states, attn_bias, attention_mask, Wq, bq, Wk, bk, Wv, bv,
           Wo, bo, c_attn):
    args = [np.asarray(a, dtype=np.float32) for a in
            (hidden_states, attn_bias, attention_mask, Wq, bq, Wk, bk, Wv, bv,
             Wo, bo, c_attn)]
    (hidden_states, attn_bias, attention_mask, Wq, bq, Wk, bk, Wv, bv,
     Wo, bo, c_attn) = args
    in_maps, with_mask = shard_inputs(hidden_states, attn_bias, attention_mask,
                                      Wq, bq, Wk, bk, Wv, bv, Wo, bo, c_attn,
                                      n_cores=N_CORES, scaling=SCALING)
    res = run_spmd(in_maps, with_mask)
    out = np.zeros((B * T, E), np.float32)
    for r in res.results:
        out += r["out"]
    out += bo[None, :]
    return out.reshape(B, T, E).astype(np.float32)


# revision 25
# speedup vs baseline: 1.2893x; 1.0225x over previous
"""OFA attention (dense_transformer) on 8 Trainium2 NeuronCores.

Sharding: heads split over cores (core c owns heads {2c, 2c+1}, both batches).
Per-core Bass/Tile program (build_attention_nc):
  phase 1 : QT/KT/VT = W_c @ hs.T (transposed projections; SCALING folded into Wq,
            c_attn folded into Wv on host; bias-add fused into PSUM drain on ScalarE)
  phase 1b: V natural = PE-transpose(VT), packed [V_A | 1 | V_B | 1] bf16
  phase 2 : per (batch, 512-token t-block), streaming 128-row s-tiles:
              ST(s,t) = K Q^T            (row-tiled K=64 matmuls)
              ST += bias.T               (bias pre-transposed to [s,t] bf16 on HOST;
                                          adds split: some s-tiles accumulate via a
                                          resident-identity matmul on PE, the rest on
                                          DVE tensor_add into an f32 staging tile)
              E = exp(ST)                (ScalarE only; exp without max-subtraction:
                                          scores+bias stay in [-8, 8])
              [O.T ; sums] += [V|1].T@E  (PV matmul also produces softmax denominators)
            per head: r = 1/sums (DVE), partition_broadcast (GpSimd), O.T drain fused
            with the r scaling (DVE tensor_mul) -> out-projection runs at full
            contract-128 (both heads in one matmul), spread across the next group's
            sp iterations; drains split DVE/GpSimd.
Host: partial outputs summed over cores + bo (the "all-reduce" of the out-projection).
"""
import sys

for _p in ("/opt/trn_rl_repo",):
    if _p not in sys.path:
        sys.path.append(_p)

import numpy as np

import concourse.bass as bass
import concourse.tile as tile
from concourse import mybir
from concourse.masks import make_identity
from concourse.bass_utils import run_bass_kernel_spmd

F32 = mybir.dt.float32
BF16 = mybir.dt.bfloat16

B, T, E, NH, D = 2, 2048, 1024, 16, 64
N_CORES = 8
HPC = NH // N_CORES
DH = HPC * D
SCALING = float(D * 2.0) ** -0.5

# Bias-add engine per unit u = sp*2 + a: 'p' = PE identity-matmul, 'v' = DVE.
# (GpSimd cannot read PSUM, so it only does broadcasts + bias DMA triggers.)
ASSIGN = ['p', 'v', 'v', 'v', 'p', 'v', 'v', 'v',
          'v', 'p', 'v', 'v', 'v', 'p', 'v', 'v']


def _waitfix(nc, limit=1):
    """This walrus build accepts at most ONE sync-wait per instruction.
    Hoist excess sem-waits onto inserted single-wait NoOps."""
    n_fixed = 0
    for bb in nc.m.functions[0].blocks:
        i = 0
        insts = bb.instructions
        while i < len(insts):
            inst = insts[i]
            si = inst.sync_info
            if si and si.on_wait and len(si.on_wait) > limit:
                extra = si.on_wait[limit:]
                si.on_wait = si.on_wait[:limit]
                for k, w in enumerate(extra):
                    nop = mybir.InstNoOp(
                        name=f"{inst.name}-waitfix{k}",
                        engine=inst.engine,
                        sync_info=mybir.SyncInfo(on_wait=[w], on_update=[]),
                        bass_nofuse=True,
                    )
                    nc.register_instruction(nop, overwrite=True)
                    insts.insert(i, nop)
                    i += 1
                n_fixed += 1
            i += 1
    return n_fixed


def build_attention_nc(B=2, T=2048, E=1024, HPC=2, D=64,
                       T_BLOCK=512, PROJ_BLOCK=512):
    """Build the per-core Bass program. Returns nc."""
    S = T
    PROJ_BLOCK = min(PROJ_BLOCK, T)
    TOK = B * T
    DH = HPC * D                      # 128
    assert DH == 128 and D == 64 and HPC == 2
    NE = E // 128                     # e-tiles
    NST = S // 128                    # s-tiles per batch
    NTB = T // T_BLOCK                # t-blocks per batch
    NJ = T_BLOCK // 128               # t-subtiles per block
    NPB = TOK // PROJ_BLOCK           # proj token blocks

    nc = bass.Bass()

    hsT = nc.declare_dram_parameter("hsT", [E, TOK], BF16, isOutput=False)
    wqT = nc.declare_dram_parameter("wqT", [E, DH], BF16, isOutput=False)
    wkT = nc.declare_dram_parameter("wkT", [E, DH], BF16, isOutput=False)
    wvT = nc.declare_dram_parameter("wvT", [E, DH], BF16, isOutput=False)
    bq = nc.declare_dram_parameter("bq", [DH, 1], F32, isOutput=False)
    bk = nc.declare_dram_parameter("bk", [DH, 1], F32, isOutput=False)
    bv = nc.declare_dram_parameter("bv", [DH, 1], F32, isOutput=False)
    woT = nc.declare_dram_parameter("woT", [DH, E], BF16, isOutput=False)
    biasT_in = nc.declare_dram_parameter(
        "biasT", [B, HPC, T // T_BLOCK, 128, S // 128, T_BLOCK], BF16,
        isOutput=False)
    out_partial = nc.declare_dram_parameter("out", [TOK, E], BF16, isOutput=True)
    rc_dram = nc.dram_tensor("rc_scratch", [B * NTB * HPC, T_BLOCK], F32)

    with tile.TileContext(nc) as tc:
        from contextlib import ExitStack
        with ExitStack() as ctx:
            consts = ctx.enter_context(tc.tile_pool(name="consts", bufs=1))
            persist = ctx.enter_context(tc.tile_pool(name="persist", bufs=1))
            bias_pool = ctx.enter_context(tc.tile_pool(name="bias_sb", bufs=2))

            i_bf = consts.tile([128, 128], BF16, tag="i_bf")
            make_identity(nc, i_bf[:])

            # weights: (E, DH) -> (128, NE, DH), bf16
            w_sb = {}
            for name, src in (("wq", wqT), ("wk", wkT), ("wv", wvT)):
                t = consts.tile([128, NE, DH], BF16, tag=name)
                nc.gpsimd.dma_start(out=t[:], in_=src.rearrange("(n p) d -> p n d", p=128))
                w_sb[name] = t
            wo_sb = consts.tile([128, E], BF16, tag="wo")
            nc.gpsimd.dma_start(out=wo_sb[:], in_=woT[:, :])
            b_sb = {}
            for name, src in (("bq", bq), ("bk", bk), ("bv", bv)):
                t = consts.tile([128, 1], F32, tag=name)
                nc.gpsimd.dma_start(out=t[:], in_=src[:, :])
                b_sb[name] = t

            # persistent activations
            QTb = [persist.tile([128, T], BF16, tag=f"QT{bb}", name=f"QT{bb}")
                   for bb in range(B)]
            KTb = [persist.tile([128, T], BF16, tag=f"KT{bb}", name=f"KT{bb}")
                   for bb in range(B)]
            VTb = [persist.tile([128, T], BF16, tag=f"VT{bb}", name=f"VT{bb}")
                   for bb in range(B)]
            V_sbb = []
            for bb in range(B):
                V_sb = persist.tile([128, T // 128, 256], BF16, tag=f"V_sb{bb}",
                                    name=f"V_sb{bb}")
                nc.vector.memset(V_sb[:, :, :], 0.0)
                nc.vector.memset(V_sb[:, :, D:D + 1], 1.0)
                nc.vector.memset(V_sb[:, :, 128 + D:128 + D + 1], 1.0)
                V_sbb.append(V_sb)

            # group list + bias prefetch plumbing (1-group emission lookahead;
            # DMAs ride the otherwise-idle GpSimd queue, transfers overlap the
            # previous group / phase 1)
            groups = [(b, tb) for b in range(B) for tb in range(NTB)]
            bias_tiles = {}

            def emit_bias_dma(gi):
                if gi >= len(groups):
                    return
                b, tb = groups[gi]
                bts = []
                for a in range(HPC):
                    t_ = bias_pool.tile([128, NST, T_BLOCK], BF16,
                                        tag=f"bias{a}", name=f"bias{b}_{tb}_{a}")
                    nc.gpsimd.dma_start(out=t_[:], in_=biasT_in[b, a, tb])
                    bts.append(t_)
                bias_tiles[gi] = bts

            # ---------------- phase 1: projections ----------------
            # all input loads ride the gpsimd queue IN ORDER (weights, then
            # hsT strips, then bias prefetch) so bias cannot starve phase 1
            with tc.tile_pool(name="hst", bufs=1) as hst_pool, \
                 tc.tile_pool(name="proj_ps", bufs=3, space="PSUM") as proj_ps:
                hstrips = {}
                for bb2 in range(B):
                    for q in range(T // 512):
                        for e in range(NE):
                            if q == 0:
                                h = hst_pool.tile([128, T], BF16,
                                                  tag=f"hst{bb2}_{e}",
                                                  name=f"hst{bb2}_{e}")
                                hstrips[(bb2, e)] = h
                            h = hstrips[(bb2, e)]
                            nc.gpsimd.dma_start(
                                out=h[:, q * 512:(q + 1) * 512],
                                in_=hsT[e * 128:(e + 1) * 128,
                                        bb2 * T + q * 512:bb2 * T + (q + 1) * 512])
                emit_bias_dma(0)
                emit_bias_dma(1)
                for pb in range(NPB):
                    t0 = pb * PROJ_BLOCK
                    bb = t0 // T
                    tloc = t0 % T
                    for name, dstl in (("wq", QTb), ("wk", KTb), ("wv", VTb)):
                        ps = proj_ps.tile([128, PROJ_BLOCK], F32, tag="proj",
                                          name=f"pps{pb}_{name}")
                        for e in range(NE):
                            nc.tensor.matmul(ps[:], w_sb[name][:, e, :],
                                             hstrips[(bb, e)][:, tloc:tloc + PROJ_BLOCK],
                                             start=(e == 0), stop=(e == NE - 1))
                        nc.vector.tensor_scalar_add(
                            out=dstl[bb][:, tloc:tloc + PROJ_BLOCK], in0=ps[:],
                            scalar1=b_sb["b" + name[1]][:])

            # ---------------- phase 1b: V natural ----------------
            with tc.tile_pool(name="vtr_ps", bufs=2, space="PSUM") as vtr_ps:
                for bb in range(B):
                    for st in range(T // 128):
                        ps = vtr_ps.tile([128, 128], BF16, tag="vtr",
                                         name=f"vtr{bb}_{st}")
                        nc.tensor.transpose(ps[:], VTb[bb][:, st * 128:(st + 1) * 128],
                                            i_bf[:])
                        nc.vector.tensor_copy(out=V_sbb[bb][:, st, 0:D],
                                              in_=ps[:, 0:D])
                        nc.vector.tensor_copy(out=V_sbb[bb][:, st, 128:128 + D],
                                              in_=ps[:, D:2 * D])

            # ---------------- phase 2: attention ----------------
            with tc.tile_pool(name="stage", bufs=6) as stage_pool, \
                 tc.tile_pool(name="e_sb", bufs=8) as e_pool, \
                 tc.tile_pool(name="ot_sb", bufs=2) as ot_sb_pool, \
                 tc.tile_pool(name="rr", bufs=6) as r_pool, \
                 tc.tile_pool(name="rb", bufs=3) as rb_pool, \
                 tc.tile_pool(name="osb", bufs=6) as out_pool, \
                 tc.tile_pool(name="st_ps", bufs=2, space="PSUM") as st_ps, \
                 tc.tile_pool(name="ot_ps", bufs=2, space="PSUM") as ot_ps, \
                 tc.tile_pool(name="wo_ps", bufs=2, space="PSUM") as wo_ps:

                # out-projection of the PREVIOUS group, emitted one (k, half)
                # item per sp iteration so the wo PSUM ring never stalls PE
                pending_wo = []   # list of closures
                os_cur = {}

                def make_wo_items(otn_raw_p, rb_p, tglob_p):
                    # item 0 applies the deferred 1/sums scaling (rb arrives
                    # with a full group of slack); items 1..8 do the
                    # out-projection matmuls + drains + stores
                    items = []

                    def scale_item(otn_raw_p=otn_raw_p, rb_p=rb_p,
                                   tglob_p=tglob_p):
                        otn_s = ot_sb_pool.tile([128, T_BLOCK], BF16,
                                                tag="otn_s",
                                                name=f"otns{tglob_p}")
                        nc.vector.tensor_mul(out=otn_s[:], in0=otn_raw_p[:],
                                             in1=rb_p[:])
                        os_cur['otn'] = otn_s
                    items.append(scale_item)
                    for k in range(NJ):
                        for h2 in range(2):
                            def item(k=k, h2=h2, tglob_p=tglob_p):
                                otn_s = os_cur['otn']
                                if h2 == 0:
                                    os_cur['t'] = out_pool.tile(
                                        [128, E], BF16, tag="osb",
                                        name=f"osb{tglob_p}_{k}")
                                os_t = os_cur['t']
                                wp = wo_ps.tile([128, 512], F32, tag="wo",
                                                name=f"wop{tglob_p}_{k}_{h2}")
                                nc.tensor.matmul(
                                    wp[:], otn_s[:, k * 128:(k + 1) * 128],
                                    wo_sb[:, h2 * 512:(h2 + 1) * 512],
                                    start=True, stop=True)
                                if h2 == 0:
                                    nc.vector.tensor_copy(
                                        out=os_t[:, 0:512], in_=wp[:])
                                else:
                                    nc.scalar.copy(
                                        out=os_t[:, 512:1024], in_=wp[:])
                                if h2 == 1:
                                    nc.sync.dma_start(
                                        out=out_partial[tglob_p + k * 128:
                                                        tglob_p + (k + 1) * 128, :],
                                        in_=os_t[:])
                            items.append(item)
                    return items

                for gi, (b, tb) in enumerate(groups):
                    tglob = b * T + tb * T_BLOCK
                    emit_bias_dma(gi + 2)
                    bt = bias_tiles.pop(gi)

                    ots = [ot_ps.tile([128, T_BLOCK], F32, tag="ot",
                                      name=f"ot{b}_{tb}_{a}") for a in range(HPC)]

                    def emit_pv(pend):
                        for a, e_sl, pst in pend:
                            nc.tensor.matmul(
                                ots[a][:],
                                V_sbb[b][:, pst, a * 128:a * 128 + 128],
                                e_sl,
                                start=(pst == 0), stop=(pst == NST - 1))

                    pendq = []
                    for sp in range(NST // 2):
                        sts = [st_ps.tile([128, 2, T_BLOCK], F32, tag="st",
                                          name=f"st{b}_{tb}_{sp}_{a}")
                               for a in range(HPC)]
                        for a in range(HPC):
                            r0 = a * D
                            pe_unit = ASSIGN[sp * 2 + a] == 'p'
                            for half in range(2):
                                st_i = sp * 2 + half
                                nc.tensor.matmul(
                                    sts[a][:, half, :],
                                    KTb[b][r0:r0 + D, st_i * 128:(st_i + 1) * 128],
                                    QTb[b][r0:r0 + D,
                                           tb * T_BLOCK:(tb + 1) * T_BLOCK],
                                    start=True, stop=not pe_unit)
                        if len(pendq) >= 2:
                            emit_pv(pendq.pop(0))
                        if pending_wo and sp >= 2:
                            pending_wo.pop(0)()
                        pend = []
                        for a in range(HPC):
                            pe_unit = ASSIGN[sp * 2 + a] == 'p'
                            e_t = e_pool.tile([128, 2, T_BLOCK], BF16, tag="et",
                                              name=f"et{b}_{tb}_{sp}_{a}")
                            if pe_unit:
                                for half in range(2):
                                    st_i = sp * 2 + half
                                    nc.tensor.matmul(
                                        sts[a][:, half, :], i_bf[:],
                                        bt[a][:, st_i, :],
                                        start=False, stop=True)
                                nc.scalar.activation(
                                    out=e_t[:], in_=sts[a][:],
                                    func=mybir.ActivationFunctionType.Exp)
                            else:
                                stg = stage_pool.tile([128, 2, T_BLOCK], F32,
                                                      tag="stg",
                                                      name=f"stg{b}_{tb}_{sp}_{a}")
                                nc.vector.tensor_add(
                                    out=stg[:], in0=sts[a][:],
                                    in1=bt[a][:, sp * 2:sp * 2 + 2, :])
                                nc.scalar.activation(
                                    out=e_t[:], in_=stg[:],
                                    func=mybir.ActivationFunctionType.Exp)
                            for half in range(2):
                                pend.append((a, e_t[:, half, :], sp * 2 + half))
                        pendq.append(pend)
                    for pend in pendq:
                        emit_pv(pend)
                    while pending_wo:
                        pending_wo.pop(0)()

                    # per head: drain O.T with a PLAIN copy (frees ot PSUM
                    # without waiting on 1/sums); r = exp(-ln(sums)) on
                    # ScalarE; rb broadcast via DRAM bounce consumed by the
                    # deferred scale item during the NEXT group
                    otn_raw = ot_sb_pool.tile([128, T_BLOCK], BF16,
                                              tag="otn_raw", name=f"otnr{b}_{tb}")
                    rb = rb_pool.tile([128, T_BLOCK], F32, tag="rb",
                                      name=f"rb{b}_{tb}")
                    for a in range(HPC):
                        ls = r_pool.tile([1, T_BLOCK], F32, tag="ls",
                                         name=f"ls{b}_{tb}_{a}")
                        nc.scalar.activation(out=ls[:], in_=ots[a][D:D + 1, :],
                                             func=mybir.ActivationFunctionType.Ln)
                        rr = r_pool.tile([1, T_BLOCK], F32, tag="rr",
                                         name=f"rr{b}_{tb}_{a}")
                        nc.scalar.activation(out=rr[:], in_=ls[:],
                                             func=mybir.ActivationFunctionType.Exp,
                                             scale=-1.0)
                        nc.vector.tensor_copy(out=otn_raw[a * D:(a + 1) * D, :],
                                              in_=ots[a][0:D, :])
                        # bounce on the gpsimd queue: it is idle at group end
                        # (bias prefetch finishes mid-group), while the sync
                        # queue's out-store backlog would delay rb by ~7us
                        idx = gi * HPC + a
                        nc.gpsimd.dma_start(out=rc_dram[idx, :], in_=rr[:])
                        src = bass.AP(rc_dram[:].tensor, idx * T_BLOCK,
                                      [[0, D], [1, T_BLOCK]])
                        nc.gpsimd.dma_start(out=rb[a * D:(a + 1) * D, :], in_=src)
                    pending_wo = make_wo_items(otn_raw, rb, tglob)
                while pending_wo:
                    pending_wo.pop(0)()
    _waitfix(nc)
    return nc


# ---------------- host-side prep ----------------

def shard_inputs(hidden_states, attn_bias, attention_mask, Wq, bq, Wk, bk, Wv, bv,
                 Wo, bo, c_attn, n_cores=8, scaling=None):
    """Build per-core input maps. Returns (in_maps, with_mask=False); the
    attention mask (when nonzero) is folded into the bias on the host."""
    import ml_dtypes
    bf16 = ml_dtypes.bfloat16
    B, T, E = hidden_states.shape
    NH = c_attn.shape[0]
    D = E // NH
    HPC = NH // n_cores
    DH = HPC * D

    hsT = np.ascontiguousarray(hidden_states.reshape(B * T, E).T).astype(bf16)
    bias4 = attn_bias.reshape(B, NH, T, T)
    if np.any(attention_mask):
        bias4 = bias4 + attention_mask.reshape(B, 1, T, T)

    if scaling is None:
        scaling = float(D * 2.0) ** -0.5

    in_maps = []
    for c in range(n_cores):
        r0 = c * DH
        sl = slice(r0, r0 + DH)
        cvec = np.repeat(c_attn[c * HPC:(c + 1) * HPC], D)
        # [B, HPC, NTB, p, st, u]: biasT6[b,a,tb,p,st,u] = bias[b, head,
        # t=tb*512+u, s=st*128+p] — matches the SBUF tile layout exactly so
        # each bias DMA reads 16KB contiguous per partition
        bh = bias4[:, c * HPC:(c + 1) * HPC]
        biasTc = np.ascontiguousarray(
            bh.reshape(B, HPC, T // 512, 512, T // 128, 128)
            .transpose(0, 1, 2, 5, 4, 3)).astype(bf16)
        m = {
            "hsT": hsT,
            "wqT": np.ascontiguousarray((Wq[sl] * scaling).T).astype(bf16),
            "wkT": np.ascontiguousarray(Wk[sl].T).astype(bf16),
            "wvT": np.ascontiguousarray((Wv[sl] * cvec[:, None]).T).astype(bf16),
            "bq": np.ascontiguousarray((bq[sl] * scaling)[:, None]).astype(np.float32),
            "bk": np.ascontiguousarray(bk[sl][:, None]).astype(np.float32),
            "bv": np.ascontiguousarray((bv[sl] * cvec)[:, None]).astype(np.float32),
            "woT": np.ascontiguousarray(Wo[:, sl].T).astype(bf16),
            "biasT": biasTc,
        }
        in_maps.append(m)
    return in_maps, False


_NC_CACHE = {}


def run_spmd(in_maps, with_mask=False, **kwargs):
    if "v2" not in _NC_CACHE:
        _NC_CACHE["v2"] = build_attention_nc(B=B, T=T, E=E, HPC=HPC, D=D)
    nc = _NC_CACHE["v2"]
    return run_bass_kernel_spmd(nc, in_maps, list(range(N_CORES)), **kwargs)


def kernel(hidden_states, attn_bias, attention_mask, Wq, bq, Wk, bk, Wv, bv,
           Wo, bo, c_attn):
    args = [np.asarray(a, dtype=np.float32) for a in
            (hidden_states, attn_bias, attention_mask, Wq, bq, Wk, bk, Wv, bv,
             Wo, bo, c_attn)]
    (hidden_states, attn_bias, attention_mask, Wq, bq, Wk, bk, Wv, bv,
     Wo, bo, c_attn) = args
    in_maps, with_mask = shard_inputs(hidden_states, attn_bias, attention_mask,
                                      Wq, bq, Wk, bk, Wv, bv, Wo, bo, c_attn,
                                      n_cores=N_CORES, scaling=SCALING)
    res = run_spmd(in_maps, with_mask)
    out = np.zeros((B * T, E), np.float32)
    for r in res.results:
        out += r["out"]
    out += bo[None, :]
    return out.reshape(B, T, E).astype(np.float32)
